# revision 6
# baseline (speedup 1.0000x reference)
"""AttentionBlock Trainium2 kernel (fp8 DoubleRow + multi-engine softmax).

Reference computation (B=16, C=512, H=W=32, n_heads=4, d_k=128):
    xs   = x.reshape(B,C,S).T            # [B, S, C],  S = 1024
    qkv  = xs @ w_proj.T + b_proj        # [B, S, 1536]
    S_   = einsum('bihd,bjhd->bijh', q, k) * d_k**-0.5
    attn = softmax(S_, axis=1)           # over the QUERY axis i (source quirk)
    res  = einsum('bijh,bjhd->bihd', attn, v)
    out  = res @ w_out.T + b_out + xs    # residual
    return out.T.reshape(B, C, H, W)

Strategy: data-parallel over batch, 2 batches per core on 8 cores.

Dtype plan (validated vs the reference in numpy, rel err ~9.5e-3 < 2e-2):
  - QKV projections: e4m3 x & weights, DoubleRow fp8 matmuls (2 k-tiles of 128
    contracted per MM, 2 output cols/cycle). QK additionally accumulates an
    x-residual correction term (xr8 = e4m3(x - e4m3(x))) to halve proj noise.
  - Q,K stored bf16; scores matmul bf16 (contraction d_k=128, no DR possible).
  - exp: ACT engine (true exp, scale=dk**-0.5, bias=-2*ln2 to keep e4m3 in
    range, accum_out gives the softmax-axis sums) for most j-tiles; for heads
    in SCH_HEADS the DVE computes exp via a Schraudolph bit-trick directly in
    e5m2 bits (y = trunc(s*A+B) as int8 == e5m2 bits of exp(s*scale)*2^-M),
    with a second DVE pass accumulating the row sums. This splits the softmax
    work across two engines.
  - AV: DoubleRow fp8, stationary v_sc = e4m3(V * 2^P / Z_j) (computed on the
    Pool engine), moving e8 tiles (e4m3 or e5m2 bits).
  - out-proj: DoubleRow e4m3 (resT stored e4m3 scaled 2^(K-P)); the residual
    (xb = x + b_out, fp32) is injected into the same psum group by a leading
    identity matmul (64*I, fp32r) so the final psum->sbuf pass is a single
    DVE multiply by 2^-K.
"""
import sys

for _p in (
    "/opt/trn_rl_repo",
    "/root/.axon_site",
    "/root/.axon_site/_ro/trn_rl_repo",
    "/root/.axon_site/_ro/pypackages",
):
    if _p not in sys.path:
        sys.path.append(_p)

import numpy as np
import ml_dtypes

B = 16
C = 512
S = 1024  # H*W
NH = 4
DK = 128
F = NH * DK  # 512
NCORES = 8
BL = B // NCORES  # batches per core
KT = C // 128  # 4 contraction tiles over channels
ST = S // 128  # 8 seq tiles
NT = S // 512  # 2 free-dim chunks of 512
SCALE = float(DK) ** -0.5

M_EXP = 2          # exp computed as exp(t - M_EXP*ln2); 2^M_EXP folded into Z
P_VSC = 7          # v_sc = V * 2^P_VSC / Z
K_RES = 6          # resT scaled 2^(K_RES-P_VSC); residual via 2^K_RES*I matmul
SCH_HEADS = (1, 3)  # heads whose exp runs on DVE (Schraudolph e5m2 bits)
ACT_CONV_TILES = (0, 1)  # qk f-tiles whose psum->bf16 conversion runs on ACT
VSC_POOL = True    # v_sc scaling on Pool engine (False: DVE)
SCH_A = float(4.0 / np.log(2.0) * SCALE)
SCH_B = float(4.0 * (15 - M_EXP) - 0.45 + 0.5)  # +0.5: DVE int8 convert truncates

_CACHE: dict = {}


def _build(repeat=1):
    """Build the kernel. repeat>1 wraps the per-call workload in an on-device
    For_i loop (timing only; amortizes the ~10ms axon dispatch)."""
    import contextlib

    import concourse.tile as tile
    from concourse import bacc, mybir

    F32 = mybir.dt.float32
    F32R = mybir.dt.float32r
    BF16 = mybir.dt.bfloat16
    E4 = mybir.dt.float8e4
    E5 = mybir.dt.float8e5
    I8 = mybir.dt.int8

    nc = bacc.Bacc("TRN2", debug=False)
    x8_d = nc.dram_tensor("x8", [BL, C, S], E4, kind="ExternalInput").ap()
    xr8_d = nc.dram_tensor("xr8", [BL, C, S], E4, kind="ExternalInput").ap()
    xb_d = nc.dram_tensor("xb", [BL, C, S], F32, kind="ExternalInput").ap()
    xb_r = xb_d.bitcast(F32R)
    wqk_d = nc.dram_tensor("wqk8", [C, 2 * F], E4, kind="ExternalInput").ap()
    wv_d = nc.dram_tensor("wv8", [C, F], E4, kind="ExternalInput").ap()
    wo_d = nc.dram_tensor("wo8", [F, C], E4, kind="ExternalInput").ap()
    consts_d = nc.dram_tensor(
        "consts", [128, 2 * NH + F + 1 + 128], F32, kind="ExternalInput"
    ).ap().bitcast(F32R)
    out_d = nc.dram_tensor("out", [BL, C, S], F32, kind="ExternalOutput").ap()

    wqk_r = wqk_d.rearrange("(k p) m -> p k m", p=128)
    wv_r = wv_d.rearrange("(k p) m -> p k m", p=128)
    wo_r = wo_d.rearrange("(k p) m -> p k m", p=128)

    with tile.TileContext(nc) as tc:
        with (
            tc.tile_pool(name="const", bufs=1) as constp,
            tc.tile_pool(name="xp", bufs=2) as xp,
            tc.tile_pool(name="qkp", bufs=2) as qkp,
            tc.tile_pool(name="vp", bufs=2) as vp,
            tc.tile_pool(name="ep", bufs=6) as ep,
            tc.tile_pool(name="scr", bufs=2) as scrp,
            tc.tile_pool(name="rp", bufs=2) as rp,
            tc.tile_pool(name="op", bufs=2) as op,
            tc.tile_pool(name="small", bufs=8) as smallp,
            tc.tile_pool(name="vs", bufs=8) as vsp,
            tc.tile_pool(name="pp", bufs=2, space="PSUM") as pp,
            tc.tile_pool(name="ps", bufs=2, space="PSUM") as ps,
            tc.tile_pool(name="pr", bufs=2, space="PSUM") as pr,
        ):
            wqk_sb = constp.tile([128, KT, 2 * F], E4)
            wv_sb = constp.tile([128, KT, F], E4)
            wo_sb = constp.tile([128, KT, C], E4)
            consts_sb = constp.tile([128, 2 * NH + F + 1 + 128], F32R)
            x8_sbs = [xp.tile([128, KT, S], E4, name=f"x8_{b}", tag="x8") for b in range(BL)]
            xr8_sbs = [xp.tile([128, KT, S], E4, name=f"xr8_{b}", tag="xr8") for b in range(BL)]
            xb_sbs = [xp.tile([128, KT, S], F32R, name=f"xb_{b}", tag="xb") for b in range(BL)]

            # DMA order: first batch's x8 + qk weights first (gate the first
            # matmuls), then the rest. One DMA per (tensor, batch) via 3D APs.
            def xre(ap_d, b):
                return ap_d[b].rearrange("(k p) s -> p k s", p=128)

            nc.sync.dma_start(out=x8_sbs[0], in_=xre(x8_d, 0))
            nc.sync.dma_start(out=wqk_sb, in_=wqk_r)
            nc.sync.dma_start(out=consts_sb, in_=consts_d)
            nc.sync.dma_start(out=xr8_sbs[0], in_=xre(xr8_d, 0))
            nc.sync.dma_start(out=wv_sb, in_=wv_r)
            for b in range(1, BL):
                nc.sync.dma_start(out=x8_sbs[b], in_=xre(x8_d, b))
                nc.sync.dma_start(out=xr8_sbs[b], in_=xre(xr8_d, b))
            nc.sync.dma_start(out=wo_sb, in_=wo_r)
            for b in range(BL):
                nc.sync.dma_start(out=xb_sbs[b], in_=xre(xb_r, b))

            b_qk = consts_sb[:, 0 : 2 * NH].bitcast(F32)
            bv_sb = consts_sb[:, 2 * NH : 2 * NH + F].bitcast(F32)
            ebias = consts_sb[:, 2 * NH + F : 2 * NH + F + 1].bitcast(F32)
            ident = consts_sb[:, 2 * NH + F + 1 :]  # [128,128] = 2^K_RES * I (fp32r)

            rep_ctx = tc.For_i(0, repeat, 1) if repeat > 1 else contextlib.nullcontext()
            with rep_ctx:
                _batches(
                    nc, tc, x8_sbs, xr8_sbs, xb_sbs, qkp, vp, ep, scrp, rp, op,
                    smallp, vsp, pp, ps, pr, wqk_sb, wv_sb, wo_sb, b_qk, bv_sb,
                    ebias, ident, out_d, F32, F32R, BF16, E4, E5, I8, mybir,
                )

    nc.compile()
    return nc


def _batches(
    nc, tc, x8_sbs, xr8_sbs, xb_sbs, qkp, vp, ep, scrp, rp, op, smallp, vsp,
    pp, ps, pr, wqk_sb, wv_sb, wo_sb, b_qk, bv_sb, ebias, ident, out_d,
    F32, F32R, BF16, E4, E5, I8, mybir,
):
    DR = mybir.MatmulPerfMode.DoubleRow
    EXP = mybir.ActivationFunctionType.Exp
    IDENT = mybir.ActivationFunctionType.Identity
    MULT = mybir.AluOpType.mult
    ADD = mybir.AluOpType.add

    for b in range(BL):
        x8_sb, xr8_sb, xb_sb = x8_sbs[b], xr8_sbs[b], xb_sbs[b]
        qk_sb = qkp.tile([128, 2 * NH, S], BF16)
        v_sb = vp.tile([128, ST, F], BF16)

        def qk_proj(t, qk_sb=qk_sb, x8_sb=x8_sb, xr8_sb=xr8_sb):
            # Q^T/K^T f-tile t: qk[:, t, s] = w_qk[:, t].T @ (x8 + xr8)
            accs = [pp.tile([128, 512], F32, name=f"qka{n}", tag="pp") for n in range(NT)]
            for kp in range(KT // 2):
                for si, src in enumerate((x8_sb, xr8_sb)):
                    for n in range(NT):
                        nc.tensor.matmul(
                            accs[n],
                            wqk_sb[:, 2 * kp : 2 * kp + 2, bass_ts(t, 128)],
                            src[:, 2 * kp : 2 * kp + 2, bass_ts(n, 512)],
                            start=(kp == 0 and si == 0),
                            stop=(kp == KT // 2 - 1 and si == 1),
                            perf_mode=DR,
                        )
            for n in range(NT):
                if t in ACT_CONV_TILES:
                    nc.scalar.activation(
                        out=qk_sb[:, t, bass_ts(n, 512)], in_=accs[n],
                        func=IDENT, bias=b_qk[:, t : t + 1],
                    )
                else:
                    nc.vector.tensor_scalar_add(
                        qk_sb[:, t, bass_ts(n, 512)], accs[n], b_qk[:, t : t + 1]
                    )

        def v_proj(st, v_sb=v_sb, x8_sb=x8_sb):
            acc = pp.tile([128, 512], F32, name="vacc", tag="pp")
            for kp in range(KT // 2):
                nc.tensor.matmul(
                    acc,
                    x8_sb[:, 2 * kp : 2 * kp + 2, bass_ts(st, 128)],
                    wv_sb[:, 2 * kp : 2 * kp + 2, :],
                    start=(kp == 0),
                    stop=(kp == KT // 2 - 1),
                    perf_mode=DR,
                )
            nc.vector.tensor_tensor(v_sb[:, st, :], acc, bv_sb, op=ADD)

        qk_proj(0)
        qk_proj(1)

        resT_sb = rp.tile([128, NH, S], E4)  # res^T * 2^(K-P): (d, head, i)
        for h in range(NH):
            sch = h in SCH_HEADS
            ssum = smallp.tile([128, 2 * ST], F32, name="ssum", tag="ssum")
            e8s = []
            for u in range(ST // 2):
                e8 = ep.tile([128, 2, S], E4, name=f"e8_{u}", tag="e8")
                e8s.append(e8)
                for i in range(2):
                    jt = 2 * u + i
                    sacc = ps.tile([128, S], F32, name="sacc", tag="sacc")
                    for n in range(NT):
                        nc.tensor.matmul(
                            sacc[:, bass_ts(n, 512)],
                            qk_sb[:, 2 * h + 1, bass_ts(jt, 128)],
                            qk_sb[:, 2 * h, bass_ts(n, 512)],
                            start=True,
                            stop=True,
                        )
                    if sch:
                        nc.vector.tensor_scalar(
                            e8[:, i, :].bitcast(I8), sacc, SCH_A, SCH_B, MULT, ADD
                        )
                        scr = scrp.tile([128, S], E5, name="scr", tag="scr")
                        nc.vector.tensor_scalar(
                            scr, e8[:, i, :].bitcast(E5), 1.0, None, MULT, ADD,
                            accum_out=ssum[:, jt : jt + 1],
                        )
                    else:
                        nc.scalar.activation(
                            out=e8[:, i, :], in_=sacc, func=EXP, scale=SCALE,
                            bias=ebias[:, 0:1], accum_out=ssum[:, jt : jt + 1],
                        )
                if u == 0 and h == 0:
                    for st in range(ST):
                        v_proj(st)
                if u == 2 and h < NH - 1:
                    qk_proj(2 * h + 2)
                    qk_proj(2 * h + 3)
            nc.vector.reciprocal(ssum[:, ST : 2 * ST], ssum[:, 0:ST])
            racc = [pr.tile([128, 512], F32, name=f"racc{n}", tag="racc") for n in range(NT)]
            for u in range(ST // 2):
                v_sc = vsp.tile([128, 2, DK], E4, name="vsc", tag="vsc")
                vsc_eng = nc.gpsimd if VSC_POOL else nc.vector
                for i in range(2):
                    jt = 2 * u + i
                    vsc_eng.tensor_scalar(
                        v_sc[:, i, :], v_sb[:, jt, bass_ts(h, DK)],
                        ssum[:, ST + jt : ST + jt + 1], float(2.0 ** P_VSC),
                        MULT, MULT,
                    )
                e8 = e8s[u]
                for n in range(NT):
                    rhs = e8[:, :, bass_ts(n, 512)]
                    nc.tensor.matmul(
                        racc[n],
                        v_sc,
                        rhs.bitcast(E5) if sch else rhs,
                        start=(u == 0),
                        stop=(u == ST // 2 - 1),
                        perf_mode=DR,
                    )
            for n in range(NT):
                nc.vector.tensor_scalar(
                    resT_sb[:, h, bass_ts(n, 512)], racc[n],
                    float(2.0 ** (K_RES - P_VSC)), None, MULT,
                )

        # ---- output projection: psum = 2^K*xb + wo8.T @ resT; out = psum*2^-K
        for ct in range(KT):
            out_t = op.tile([128, S], F32)
            accs = [pp.tile([128, 512], F32, name=f"oa{n}", tag="pp") for n in range(NT)]
            for n in range(NT):
                nc.tensor.matmul(
                    accs[n],
                    ident,
                    xb_sb[:, ct, bass_ts(n, 512)],
                    start=True,
                    stop=False,
                )
                for kp in range(KT // 2):
                    nc.tensor.matmul(
                        accs[n],
                        wo_sb[:, 2 * kp : 2 * kp + 2, bass_ts(ct, 128)],
                        resT_sb[:, 2 * kp : 2 * kp + 2, bass_ts(n, 512)],
                        start=False,
                        stop=(kp == KT // 2 - 1),
                        perf_mode=DR,
                    )
                nc.vector.tensor_scalar(
                    out_t[:, bass_ts(n, 512)], accs[n],
                    float(2.0 ** (-K_RES)), None, MULT,
                )
                nc.sync.dma_start(
                    out=out_d[b, bass_ts(ct, 128), bass_ts(n, 512)],
                    in_=out_t[:, bass_ts(n, 512)],
                )


def bass_ts(i, size):
    import concourse.bass as bass

    return bass.ts(i, size)


def _prep_inputs(x, w_proj, b_proj, w_out, b_out):
    """Host-side quantization + reshaping into the layouts the kernel expects."""
    E4 = ml_dtypes.float8_e4m3
    x_f = np.ascontiguousarray(np.asarray(x, np.float32).reshape(B, C, S))
    b_out = np.asarray(b_out, np.float32)
    xb = np.ascontiguousarray(x_f + b_out[None, :, None])
    x8 = x_f.astype(E4)
    xr8 = np.ascontiguousarray(x_f - x8.astype(np.float32)).astype(E4)
    x8 = np.ascontiguousarray(x8)

    wT = np.asarray(w_proj, np.float32).T  # [C, 3F]
    w_qkT = np.concatenate(
        [wT[:, h * 384 : h * 384 + 256] for h in range(NH)], axis=1
    )  # [C, 2F]; col tile t=2h -> q_h, t=2h+1 -> k_h
    w_vT = np.concatenate(
        [wT[:, h * 384 + 256 : h * 384 + 384] for h in range(NH)], axis=1
    )  # [C, F]
    w_outT = np.asarray(w_out, np.float32).T  # [F, C]
    wqk8 = np.ascontiguousarray(w_qkT).astype(E4)
    wv8 = np.ascontiguousarray(w_vT).astype(E4)
    wo8 = np.ascontiguousarray(w_outT).astype(E4)

    b_proj = np.asarray(b_proj, np.float32)
    b_qk = np.stack(
        [
            b_proj[h * 384 + half * 128 : h * 384 + half * 128 + 128]
            for h in range(NH)
            for half in range(2)
        ],
        axis=1,
    )  # [128, 2*NH]
    b_v = np.concatenate([b_proj[h * 384 + 256 : h * 384 + 384] for h in range(NH)])
    bv_bcast = np.broadcast_to(b_v, (128, F))
    eb = np.full((128, 1), -M_EXP * np.log(2.0), np.float32)
    ident = np.eye(128, dtype=np.float32) * float(2.0 ** K_RES)
    consts = np.ascontiguousarray(
        np.concatenate([b_qk, bv_bcast, eb, ident], axis=1), dtype=np.float32
    )
    return x8, xr8, xb, wqk8, wv8, wo8, consts


def kernel(x, w_proj, b_proj, w_out, b_out, n_heads):
    from concourse.bass_utils import run_bass_kernel_spmd

    assert int(n_heads) == NH
    x8, xr8, xb, wqk8, wv8, wo8, consts = _prep_inputs(x, w_proj, b_proj, w_out, b_out)

    if "nc" not in _CACHE:
        _CACHE["nc"] = _build()
    nc = _CACHE["nc"]

    in_maps = [
        {
            "x8": np.ascontiguousarray(x8[c * BL : (c + 1) * BL]),
            "xr8": np.ascontiguousarray(xr8[c * BL : (c + 1) * BL]),
            "xb": np.ascontiguousarray(xb[c * BL : (c + 1) * BL]),
            "wqk8": wqk8,
            "wv8": wv8,
            "wo8": wo8,
            "consts": consts,
        }
        for c in range(NCORES)
    ]
    res = run_bass_kernel_spmd(nc, in_maps, list(range(NCORES)))
    out = np.concatenate([res.results[c]["out"] for c in range(NCORES)], axis=0)
    return out.reshape(B, C, 32, 32)


# revision 10
# speedup vs baseline: 1.0965x; 1.0965x over previous
"""AttentionBlock Trainium2 kernel (fp8 DoubleRow + multi-engine softmax).

Reference computation (B=16, C=512, H=W=32, n_heads=4, d_k=128):
    xs   = x.reshape(B,C,S).T            # [B, S, C],  S = 1024
    qkv  = xs @ w_proj.T + b_proj        # [B, S, 1536]
    S_   = einsum('bihd,bjhd->bijh', q, k) * d_k**-0.5
    attn = softmax(S_, axis=1)           # over the QUERY axis i (source quirk)
    res  = einsum('bijh,bjhd->bihd', attn, v)
    out  = res @ w_out.T + b_out + xs    # residual
    return out.T.reshape(B, C, H, W)

Strategy: data-parallel over batch, 2 batches per core on 8 cores.

Dtype plan (validated vs the reference in numpy, rel err ~9.4e-3 < 2e-2):
  - QKV projections: e4m3 x & weights, DoubleRow fp8 matmuls (2 k-tiles of 128
    contracted per MM, 2 output cols/cycle). QK additionally accumulates an
    x-residual correction term (xr8 = e4m3(x - e4m3(x))) to halve proj noise.
  - Q,K stored bf16; scores matmul bf16 (contraction d_k=128, no DR possible).
  - exp on ACT: scale=dk**-0.5, bias=-2*ln2 keeps e4m3 in range; accum_out
    produces the softmax-axis sums. (Optionally SCH_HEADS run exp on DVE via a
    Schraudolph bit-trick in e5m2; measured slower on HW - off by default.)
  - AV: DoubleRow fp8, stationary v_sc = e4m3(V * 2^P / Z_j) (Pool engine),
    moving e8 tiles.
  - out-proj: DoubleRow e4m3 (resT stored e4m3 scaled 2^(K-P)); the residual
    (xb = x + b_out, fp32) is injected into the same psum group by a leading
    identity matmul (2^K*I, fp32r) so the final psum->sbuf pass is a single
    multiply by 2^-K.

All psum->sbuf evacuation passes use [128,1024] (2-bank) tiles to halve the
per-instruction overhead on DVE/ACT, which HW timing showed dominates.
"""
import sys

for _p in (
    "/opt/trn_rl_repo",
    "/root/.axon_site",
    "/root/.axon_site/_ro/trn_rl_repo",
    "/root/.axon_site/_ro/pypackages",
):
    if _p not in sys.path:
        sys.path.append(_p)

import numpy as np
import ml_dtypes

B = 16
C = 512
S = 1024  # H*W
NH = 4
DK = 128
F = NH * DK  # 512
NCORES = 8
BL = B // NCORES  # batches per core
KT = C // 128  # 4 contraction tiles over channels
ST = S // 128  # 8 seq tiles
NT = S // 512  # 2 free-dim chunks of 512
SCALE = float(DK) ** -0.5

M_EXP = 2          # exp computed as exp(t - M_EXP*ln2); 2^M_EXP folded into Z
P_VSC = 7          # v_sc = V * 2^P_VSC / Z
K_RES = 6          # resT scaled 2^(K_RES-P_VSC); residual via 2^K_RES*I matmul
SCH_HEADS = ()     # heads whose exp runs on DVE (Schraudolph e5m2 bits)
ACT_CONV_TILES = ()  # qk f-tiles whose psum->bf16 conversion runs on ACT
OUT_CONV_ACT = ()  # out-proj c-tiles whose psum->f32 conversion runs on ACT
RES_CONV_ACT = ()  # heads whose racc->resT conversion runs on ACT
VSC_POOL = True    # v_sc scaling on Pool engine (False: DVE)
USE_FEEDBACK = True  # QK proj accumulates xr8 correction term
SCH_A = float(4.0 / np.log(2.0) * SCALE)
SCH_B = float(4.0 * (15 - M_EXP) - 0.45 + 0.5)  # +0.5: DVE int8 convert truncates

_CACHE: dict = {}


def _build(repeat=1):
    """Build the kernel. repeat>1 wraps the per-call workload in an on-device
    For_i loop (timing only; amortizes the ~10ms axon dispatch)."""
    import contextlib

    import concourse.tile as tile
    from concourse import bacc, mybir

    F32 = mybir.dt.float32
    F32R = mybir.dt.float32r
    BF16 = mybir.dt.bfloat16
    E4 = mybir.dt.float8e4
    E5 = mybir.dt.float8e5
    I8 = mybir.dt.int8

    nc = bacc.Bacc("TRN2", debug=False)
    x8_d = nc.dram_tensor("x8", [BL, C, S], E4, kind="ExternalInput").ap()
    xr8_d = nc.dram_tensor("xr8", [BL, C, S], E4, kind="ExternalInput").ap()
    xb_d = nc.dram_tensor("xb", [BL, C, S], F32, kind="ExternalInput").ap()
    xb_r = xb_d.bitcast(F32R)
    wqk_d = nc.dram_tensor("wqk8", [C, 2 * F], E4, kind="ExternalInput").ap()
    wv_d = nc.dram_tensor("wv8", [C, F], E4, kind="ExternalInput").ap()
    wo_d = nc.dram_tensor("wo8", [F, C], E4, kind="ExternalInput").ap()
    NCONST = 2 * NH + 2 * F + 1 + 128
    consts_d = nc.dram_tensor(
        "consts", [128, NCONST], F32, kind="ExternalInput"
    ).ap().bitcast(F32R)
    out_d = nc.dram_tensor("out", [BL, C, S], F32, kind="ExternalOutput").ap()

    wqk_r = wqk_d.rearrange("(k p) m -> p k m", p=128)
    wv_r = wv_d.rearrange("(k p) m -> p k m", p=128)
    wo_r = wo_d.rearrange("(k p) m -> p k m", p=128)

    with tile.TileContext(nc) as tc:
        with (
            tc.tile_pool(name="const", bufs=1) as constp,
            tc.tile_pool(name="xp", bufs=2) as xp,
            tc.tile_pool(name="qkp", bufs=2) as qkp,
            tc.tile_pool(name="vp", bufs=2) as vp,
            tc.tile_pool(name="ep", bufs=6) as ep,
            tc.tile_pool(name="scr", bufs=2) as scrp,
            tc.tile_pool(name="rp", bufs=2) as rp,
            tc.tile_pool(name="op", bufs=2) as op,
            tc.tile_pool(name="small", bufs=8) as smallp,
            tc.tile_pool(name="vs", bufs=8) as vsp,
            # psum: one pool, per-tag rings: "wp" (proj/out-proj) 1x[128,1024],
            # "sacc" 2x[128,1024], "racc" 1x[128,1024] -> 2+4+2 = 8 banks.
            tc.tile_pool(name="pz", bufs=1, space="PSUM") as pz,
        ):
            wqk_sb = constp.tile([128, KT, 2 * F], E4)
            wv_sb = constp.tile([128, KT, F], E4)
            wo_sb = constp.tile([128, KT, C], E4)
            consts_sb = constp.tile([128, NCONST], F32R)
            x8_sbs = [xp.tile([128, KT, S], E4, name=f"x8_{b}", tag="x8") for b in range(BL)]
            xr8_sbs = [xp.tile([128, KT, S], E4, name=f"xr8_{b}", tag="xr8") for b in range(BL)]
            xb_sbs = [xp.tile([128, KT, S], F32R, name=f"xb_{b}", tag="xb") for b in range(BL)]

            # DMA order: first batch's x8 + qk weights first (gate the first
            # matmuls), then the rest. One DMA per (tensor, batch) via 3D APs.
            def xre(ap_d, b):
                return ap_d[b].rearrange("(k p) s -> p k s", p=128)

            nc.sync.dma_start(out=x8_sbs[0], in_=xre(x8_d, 0))
            nc.sync.dma_start(out=wqk_sb, in_=wqk_r)
            nc.sync.dma_start(out=consts_sb, in_=consts_d)
            nc.sync.dma_start(out=xr8_sbs[0], in_=xre(xr8_d, 0))
            nc.sync.dma_start(out=wv_sb, in_=wv_r)
            for b in range(1, BL):
                nc.sync.dma_start(out=x8_sbs[b], in_=xre(x8_d, b))
                nc.sync.dma_start(out=xr8_sbs[b], in_=xre(xr8_d, b))
            nc.sync.dma_start(out=wo_sb, in_=wo_r)
            for b in range(BL):
                nc.sync.dma_start(out=xb_sbs[b], in_=xre(xb_r, b))

            b_qk = consts_sb[:, 0 : 2 * NH].bitcast(F32)
            bv2_sb = consts_sb[:, 2 * NH : 2 * NH + 2 * F].bitcast(F32)
            ebias = consts_sb[:, 2 * NH + 2 * F : 2 * NH + 2 * F + 1].bitcast(F32)
            ident = consts_sb[:, 2 * NH + 2 * F + 1 :]  # [128,128] = 2^K_RES*I (fp32r)

            rep_ctx = tc.For_i(0, repeat, 1) if repeat > 1 else contextlib.nullcontext()
            with rep_ctx:
                _batches(
                    nc, tc, x8_sbs, xr8_sbs, xb_sbs, qkp, vp, ep, scrp, rp, op,
                    smallp, vsp, pz, wqk_sb, wv_sb, wo_sb, b_qk, bv2_sb,
                    ebias, ident, out_d, F32, F32R, BF16, E4, E5, I8, mybir,
                )

    nc.compile()
    return nc


def _batches(
    nc, tc, x8_sbs, xr8_sbs, xb_sbs, qkp, vp, ep, scrp, rp, op, smallp, vsp,
    pz, wqk_sb, wv_sb, wo_sb, b_qk, bv2_sb, ebias, ident, out_d,
    F32, F32R, BF16, E4, E5, I8, mybir,
):
    DR = mybir.MatmulPerfMode.DoubleRow
    EXP = mybir.ActivationFunctionType.Exp
    IDENT = mybir.ActivationFunctionType.Identity
    MULT = mybir.AluOpType.mult
    ADD = mybir.AluOpType.add

    pending_out = [None]  # emits one c-tile of the previous batch's out-proj

    def emit_pending(k=1):
        for _ in range(k):
            if pending_out[0]:
                pending_out[0]()

    for b in range(BL):
        x8_sb, xr8_sb, xb_sb = x8_sbs[b], xr8_sbs[b], xb_sbs[b]
        qk_sb = qkp.tile([128, 2 * NH, S], BF16)
        v_sb = vp.tile([128, ST // 2, 2 * F], BF16)  # [j_lo, st-pair, (st-par, f)]

        def qk_proj(t, qk_sb=qk_sb, x8_sb=x8_sb, xr8_sb=xr8_sb):
            # Q^T/K^T f-tile t: qk[:, t, s] = w_qk[:, t].T @ (x8 + xr8)
            acc = pz.tile([128, S], F32, name="qka", tag="wp", bufs=1)
            srcs = (x8_sb, xr8_sb) if USE_FEEDBACK else (x8_sb,)
            for kp in range(KT // 2):
                for si, src in enumerate(srcs):
                    for n in range(NT):
                        nc.tensor.matmul(
                            acc[:, bass_ts(n, 512)],
                            wqk_sb[:, 2 * kp : 2 * kp + 2, bass_ts(t, 128)],
                            src[:, 2 * kp : 2 * kp + 2, bass_ts(n, 512)],
                            start=(kp == 0 and si == 0),
                            stop=(kp == KT // 2 - 1 and si == len(srcs) - 1),
                            perf_mode=DR,
                        )
            if t in ACT_CONV_TILES:
                nc.scalar.activation(
                    out=qk_sb[:, t, :], in_=acc, func=IDENT,
                    bias=b_qk[:, t : t + 1],
                )
            else:
                nc.vector.tensor_scalar_add(qk_sb[:, t, :], acc, b_qk[:, t : t + 1])

        def v_proj(u, v_sb=v_sb, x8_sb=x8_sb):
            # V rows for st-pair u: v_sb[:, u, i*F + f] = V[j=128*(2u+i)+p, f]
            acc = pz.tile([128, S], F32, name="vacc", tag="wp", bufs=1)
            for i in range(2):
                st = 2 * u + i
                for kp in range(KT // 2):
                    nc.tensor.matmul(
                        acc[:, bass_ts(i, 512)],
                        x8_sb[:, 2 * kp : 2 * kp + 2, bass_ts(st, 128)],
                        wv_sb[:, 2 * kp : 2 * kp + 2, :],
                        start=(kp == 0),
                        stop=(kp == KT // 2 - 1),
                        perf_mode=DR,
                    )
            nc.vector.tensor_tensor(v_sb[:, u, :], acc, bv2_sb, op=ADD)

        qk_proj(0)
        emit_pending(2)
        qk_proj(1)
        emit_pending(2)

        resT_sb = rp.tile([128, NH, S], E4)  # res^T * 2^(K-P): (d, head, i)
        for h in range(NH):
            sch = h in SCH_HEADS
            ssum = smallp.tile([128, 2 * ST], F32, name="ssum", tag="ssum")
            racc = pz.tile([128, S], F32, name="racc", tag="racc", bufs=1)
            e8s = []
            for u in range(ST // 2):
                e8 = ep.tile([128, 2, S], E4, name=f"e8_{u}", tag="e8")
                e8s.append(e8)
                for i in range(2):
                    jt = 2 * u + i
                    sacc = pz.tile([128, S], F32, name="sacc", tag="sacc", bufs=2)
                    for n in range(NT):
                        nc.tensor.matmul(
                            sacc[:, bass_ts(n, 512)],
                            qk_sb[:, 2 * h + 1, bass_ts(jt, 128)],
                            qk_sb[:, 2 * h, bass_ts(n, 512)],
                            start=True,
                            stop=True,
                        )
                    if sch:
                        nc.vector.tensor_scalar(
                            e8[:, i, :].bitcast(I8), sacc, SCH_A, SCH_B, MULT, ADD
                        )
                        scr = scrp.tile([128, S], E5, name="scr", tag="scr")
                        nc.vector.tensor_scalar(
                            scr, e8[:, i, :].bitcast(E5), 1.0, None, MULT, ADD,
                            accum_out=ssum[:, jt : jt + 1],
                        )
                    else:
                        nc.scalar.activation(
                            out=e8[:, i, :], in_=sacc, func=EXP, scale=SCALE,
                            bias=ebias[:, 0:1], accum_out=ssum[:, jt : jt + 1],
                        )
                if h == 0:
                    v_proj(u)
                    if u in (1, 2):
                        qk_proj(u + 1)  # t=2, t=3 for head 1
                elif h < NH - 1 and u in (1, 3):
                    qk_proj(2 * h + 2 + (u == 3))  # t=4..7 for heads 2,3
            nc.vector.reciprocal(ssum[:, ST : 2 * ST], ssum[:, 0:ST])
            for u in range(ST // 2):
                v_sc = vsp.tile([128, 2, DK], E4, name="vsc", tag="vsc")
                vsc_eng = nc.gpsimd if VSC_POOL else nc.vector
                for i in range(2):
                    jt = 2 * u + i
                    vsc_eng.tensor_scalar(
                        v_sc[:, i, :],
                        v_sb[:, u, bass_ts(i, 512)][:, bass_ts(h, DK)],
                        ssum[:, ST + jt : ST + jt + 1], float(2.0 ** P_VSC),
                        MULT, MULT,
                    )
                e8 = e8s[u]
                for n in range(NT):
                    rhs = e8[:, :, bass_ts(n, 512)]
                    nc.tensor.matmul(
                        racc[:, bass_ts(n, 512)],
                        v_sc,
                        rhs.bitcast(E5) if sch else rhs,
                        start=(u == 0),
                        stop=(u == ST // 2 - 1),
                        perf_mode=DR,
                    )
            if h in RES_CONV_ACT:
                nc.scalar.mul(resT_sb[:, h, :], racc, float(2.0 ** (K_RES - P_VSC)))
            else:
                nc.vector.tensor_scalar(
                    resT_sb[:, h, :], racc, float(2.0 ** (K_RES - P_VSC)), None, MULT,
                )

        # ---- output projection: psum = 2^K*xb + wo8.T @ resT; out = psum*2^-K
        # Emitted lazily (interleaved into the NEXT batch's projections) so the
        # PE stream isn't serialized on the shared wide-psum ring.
        def make_out_proj(b=b, xb_sb=xb_sb, resT_sb=resT_sb):
            state = {"ct": 0}

            def emit_one():
                ct = state["ct"]
                if ct >= KT:
                    return
                state["ct"] = ct + 1
                out_t = op.tile([128, S], F32, name="out_t", tag="out")
                acc = pz.tile([128, S], F32, name="oacc", tag="wp", bufs=1)
                for n in range(NT):
                    nc.tensor.matmul(
                        acc[:, bass_ts(n, 512)],
                        ident,
                        xb_sb[:, ct, bass_ts(n, 512)],
                        start=True,
                        stop=False,
                    )
                    for kp in range(KT // 2):
                        nc.tensor.matmul(
                            acc[:, bass_ts(n, 512)],
                            wo_sb[:, 2 * kp : 2 * kp + 2, bass_ts(ct, 128)],
                            resT_sb[:, 2 * kp : 2 * kp + 2, bass_ts(n, 512)],
                            start=False,
                            stop=(kp == KT // 2 - 1),
                            perf_mode=DR,
                        )
                if ct in OUT_CONV_ACT:
                    nc.scalar.mul(out_t, acc, float(2.0 ** (-K_RES)))
                else:
                    nc.vector.tensor_scalar(
                        out_t, acc, float(2.0 ** (-K_RES)), None, MULT,
                    )
                nc.sync.dma_start(out=out_d[b, :, :].rearrange("(k p) s -> p k s", p=128)[:, ct, :], in_=out_t)

            return emit_one

        pending_out[0] = make_out_proj()

    # drain the last batch's out-proj
    for _ in range(KT):
        pending_out[0]()


def bass_ts(i, size):
    import concourse.bass as bass

    return bass.ts(i, size)


def _prep_inputs(x, w_proj, b_proj, w_out, b_out):
    """Host-side quantization + reshaping into the layouts the kernel expects."""
    E4 = ml_dtypes.float8_e4m3
    x_f = np.ascontiguousarray(np.asarray(x, np.float32).reshape(B, C, S))
    b_out = np.asarray(b_out, np.float32)
    xb = np.ascontiguousarray(x_f + b_out[None, :, None])
    x8 = x_f.astype(E4)
    xr8 = np.ascontiguousarray(x_f - x8.astype(np.float32)).astype(E4)
    x8 = np.ascontiguousarray(x8)

    wT = np.asarray(w_proj, np.float32).T  # [C, 3F]
    w_qkT = np.concatenate(
        [wT[:, h * 384 : h * 384 + 256] for h in range(NH)], axis=1
    )  # [C, 2F]; col tile t=2h -> q_h, t=2h+1 -> k_h
    w_vT = np.concatenate(
        [wT[:, h * 384 + 256 : h * 384 + 384] for h in range(NH)], axis=1
    )  # [C, F]
    w_outT = np.asarray(w_out, np.float32).T  # [F, C]
    wqk8 = np.ascontiguousarray(w_qkT).astype(E4)
    wv8 = np.ascontiguousarray(w_vT).astype(E4)
    wo8 = np.ascontiguousarray(w_outT).astype(E4)

    b_proj = np.asarray(b_proj, np.float32)
    b_qk = np.stack(
        [
            b_proj[h * 384 + half * 128 : h * 384 + half * 128 + 128]
            for h in range(NH)
            for half in range(2)
        ],
        axis=1,
    )  # [128, 2*NH]
    b_v = np.concatenate([b_proj[h * 384 + 256 : h * 384 + 384] for h in range(NH)])
    bv2 = np.broadcast_to(np.concatenate([b_v, b_v]), (128, 2 * F))
    eb = np.full((128, 1), -M_EXP * np.log(2.0), np.float32)
    ident = np.eye(128, dtype=np.float32) * float(2.0 ** K_RES)
    consts = np.ascontiguousarray(
        np.concatenate([b_qk, bv2, eb, ident], axis=1), dtype=np.float32
    )
    return x8, xr8, xb, wqk8, wv8, wo8, consts


def kernel(x, w_proj, b_proj, w_out, b_out, n_heads):
    from concourse.bass_utils import run_bass_kernel_spmd

    assert int(n_heads) == NH
    x8, xr8, xb, wqk8, wv8, wo8, consts = _prep_inputs(x, w_proj, b_proj, w_out, b_out)

    if "nc" not in _CACHE:
        _CACHE["nc"] = _build()
    nc = _CACHE["nc"]

    in_maps = [
        {
            "x8": np.ascontiguousarray(x8[c * BL : (c + 1) * BL]),
            "xr8": np.ascontiguousarray(xr8[c * BL : (c + 1) * BL]),
            "xb": np.ascontiguousarray(xb[c * BL : (c + 1) * BL]),
            "wqk8": wqk8,
            "wv8": wv8,
            "wo8": wo8,
            "consts": consts,
        }
        for c in range(NCORES)
    ]
    res = run_bass_kernel_spmd(nc, in_maps, list(range(NCORES)))
    out = np.concatenate([res.results[c]["out"] for c in range(NCORES)], axis=0)
    return out.reshape(B, C, 32, 32)


# revision 12
# speedup vs baseline: 1.2318x; 1.1234x over previous
"""AttentionBlock Trainium2 kernel (fp8 DoubleRow + multi-engine softmax).

Reference computation (B=16, C=512, H=W=32, n_heads=4, d_k=128):
    xs   = x.reshape(B,C,S).T            # [B, S, C],  S = 1024
    qkv  = xs @ w_proj.T + b_proj        # [B, S, 1536]
    S_   = einsum('bihd,bjhd->bijh', q, k) * d_k**-0.5
    attn = softmax(S_, axis=1)           # over the QUERY axis i (source quirk)
    res  = einsum('bijh,bjhd->bihd', attn, v)
    out  = res @ w_out.T + b_out + xs    # residual
    return out.T.reshape(B, C, H, W)

Strategy: data-parallel over batch, 2 batches per core on 8 cores.

Dtype plan (validated vs the reference in numpy, rel err ~9.4e-3 < 2e-2):
  - QKV projections: e4m3 x & weights, DoubleRow fp8 matmuls (2 k-tiles of 128
    contracted per MM, 2 output cols/cycle). QK additionally accumulates an
    x-residual correction term (xr8 = e4m3(x - e4m3(x))) to halve proj noise.
  - Q,K stored bf16; scores matmul bf16 (contraction d_k=128, no DR possible).
  - exp on ACT: scale=dk**-0.5, bias=-2*ln2 keeps e4m3 in range; accum_out
    produces the softmax-axis sums. (Optionally SCH_HEADS run exp on DVE via a
    Schraudolph bit-trick in e5m2; measured slower on HW - off by default.)
  - AV: DoubleRow fp8, stationary v_sc = e4m3(V * 2^P / Z_j) (Pool engine),
    moving e8 tiles.
  - out-proj: DoubleRow e4m3 (resT stored e4m3 scaled 2^(K-P)); the residual
    (xb = x + b_out, fp32) is injected into the same psum group by a leading
    identity matmul (2^K*I, fp32r) so the final psum->sbuf pass is a single
    multiply by 2^-K.

All psum->sbuf evacuation passes use [128,1024] (2-bank) tiles to halve the
per-instruction overhead on DVE/ACT, which HW timing showed dominates.
"""
import sys

for _p in (
    "/opt/trn_rl_repo",
    "/root/.axon_site",
    "/root/.axon_site/_ro/trn_rl_repo",
    "/root/.axon_site/_ro/pypackages",
):
    if _p not in sys.path:
        sys.path.append(_p)

import numpy as np
import ml_dtypes

B = 16
C = 512
S = 1024  # H*W
NH = 4
DK = 128
F = NH * DK  # 512
NCORES = 8
BL = B // NCORES  # batches per core
KT = C // 128  # 4 contraction tiles over channels
ST = S // 128  # 8 seq tiles
NT = S // 512  # 2 free-dim chunks of 512
SCALE = float(DK) ** -0.5

M_EXP = 2          # exp computed as exp(t - M_EXP*ln2); 2^M_EXP folded into Z
P_VSC = 7          # v_sc = V * 2^P_VSC / Z
K_RES = 6          # resT scaled 2^(K_RES-P_VSC); residual via 2^K_RES*I matmul
SCH_HEADS = ()     # heads whose exp runs on DVE (Schraudolph e5m2 bits)
ACT_CONV_TILES = ()  # qk f-tiles whose psum->bf16 conversion runs on ACT
OUT_CONV_ACT = ()  # out-proj c-tiles whose psum->f32 conversion runs on ACT
RES_CONV_ACT = ()  # heads whose racc->resT conversion runs on ACT
VSC_POOL = True    # v_sc scaling on Pool engine (False: DVE)
USE_FEEDBACK = True  # QK proj accumulates xr8 correction term
NO_ACCUM_PROBE = False  # TIMING PROBE ONLY: drop accum_out (wrong results)
NO_EXP_PROBE = False    # TIMING PROBE ONLY: skip exp entirely (wrong results)
SCH_A = float(4.0 / np.log(2.0) * SCALE)
SCH_B = float(4.0 * (15 - M_EXP) - 0.45 + 0.5)  # +0.5: DVE int8 convert truncates

_CACHE: dict = {}


def _build(repeat=1):
    """Build the kernel. repeat>1 wraps the per-call workload in an on-device
    For_i loop (timing only; amortizes the ~10ms axon dispatch)."""
    import contextlib

    import concourse.tile as tile
    from concourse import bacc, mybir

    F32 = mybir.dt.float32
    F32R = mybir.dt.float32r
    BF16 = mybir.dt.bfloat16
    E4 = mybir.dt.float8e4
    E5 = mybir.dt.float8e5
    I8 = mybir.dt.int8

    nc = bacc.Bacc("TRN2", debug=False)
    x8_d = nc.dram_tensor("x8", [BL, C, S], E4, kind="ExternalInput").ap()
    xr8_d = nc.dram_tensor("xr8", [BL, C, S], E4, kind="ExternalInput").ap()
    xb_d = nc.dram_tensor("xb", [BL, C, S], F32, kind="ExternalInput").ap()
    xb_r = xb_d.bitcast(F32R)
    wqk_d = nc.dram_tensor("wqk8", [C, 2 * F], E4, kind="ExternalInput").ap()
    wv_d = nc.dram_tensor("wv8", [C, F], E4, kind="ExternalInput").ap()
    wo_d = nc.dram_tensor("wo8", [F, C], E4, kind="ExternalInput").ap()
    NCONST = 2 * NH + 2 * F + 1 + 128
    consts_d = nc.dram_tensor(
        "consts", [128, NCONST], F32, kind="ExternalInput"
    ).ap().bitcast(F32R)
    out_d = nc.dram_tensor("out", [BL, C, S], F32, kind="ExternalOutput").ap()

    wqk_r = wqk_d.rearrange("(k p) m -> p k m", p=128)
    wv_r = wv_d.rearrange("(k p) m -> p k m", p=128)
    wo_r = wo_d.rearrange("(k p) m -> p k m", p=128)

    with tile.TileContext(nc) as tc:
        with (
            tc.tile_pool(name="const", bufs=1) as constp,
            tc.tile_pool(name="xp", bufs=2) as xp,
            tc.tile_pool(name="qkp", bufs=2) as qkp,
            tc.tile_pool(name="vp", bufs=2) as vp,
            tc.tile_pool(name="ep", bufs=6) as ep,
            tc.tile_pool(name="scr", bufs=2) as scrp,
            tc.tile_pool(name="rp", bufs=2) as rp,
            tc.tile_pool(name="op", bufs=2) as op,
            tc.tile_pool(name="small", bufs=8) as smallp,
            tc.tile_pool(name="vs", bufs=8) as vsp,
            # psum: one pool, per-tag rings: "wp" (proj/out-proj) 1x[128,1024],
            # "sacc" 2x[128,1024], "racc" 1x[128,1024] -> 2+4+2 = 8 banks.
            tc.tile_pool(name="pz", bufs=1, space="PSUM") as pz,
        ):
            wqk_sb = constp.tile([128, KT, 2 * F], E4)
            wv_sb = constp.tile([128, KT, F], E4)
            wo_sb = constp.tile([128, KT, C], E4)
            consts_sb = constp.tile([128, NCONST], F32R)
            x8_sbs = [xp.tile([128, KT, S], E4, name=f"x8_{b}", tag="x8") for b in range(BL)]
            xr8_sbs = [xp.tile([128, KT, S], E4, name=f"xr8_{b}", tag="xr8") for b in range(BL)]
            xb_sbs = [xp.tile([128, KT, S], F32R, name=f"xb_{b}", tag="xb") for b in range(BL)]

            # DMA order: first batch's x8 + qk weights first (gate the first
            # matmuls), then the rest. One DMA per (tensor, batch) via 3D APs.
            def xre(ap_d, b):
                return ap_d[b].rearrange("(k p) s -> p k s", p=128)

            nc.sync.dma_start(out=x8_sbs[0], in_=xre(x8_d, 0))
            nc.sync.dma_start(out=wqk_sb, in_=wqk_r)
            nc.sync.dma_start(out=consts_sb, in_=consts_d)
            nc.sync.dma_start(out=xr8_sbs[0], in_=xre(xr8_d, 0))
            nc.sync.dma_start(out=wv_sb, in_=wv_r)
            for b in range(1, BL):
                nc.sync.dma_start(out=x8_sbs[b], in_=xre(x8_d, b))
                nc.sync.dma_start(out=xr8_sbs[b], in_=xre(xr8_d, b))
            nc.sync.dma_start(out=wo_sb, in_=wo_r)
            for b in range(BL):
                nc.sync.dma_start(out=xb_sbs[b], in_=xre(xb_r, b))

            b_qk = consts_sb[:, 0 : 2 * NH].bitcast(F32)
            bv2_sb = consts_sb[:, 2 * NH : 2 * NH + 2 * F].bitcast(F32)
            ebias = consts_sb[:, 2 * NH + 2 * F : 2 * NH + 2 * F + 1].bitcast(F32)
            ident = consts_sb[:, 2 * NH + 2 * F + 1 :]  # [128,128] = 2^K_RES*I (fp32r)

            rep_ctx = tc.For_i(0, repeat, 1) if repeat > 1 else contextlib.nullcontext()
            with rep_ctx:
                _batches(
                    nc, tc, x8_sbs, xr8_sbs, xb_sbs, qkp, vp, ep, scrp, rp, op,
                    smallp, vsp, pz, wqk_sb, wv_sb, wo_sb, b_qk, bv2_sb,
                    ebias, ident, out_d, F32, F32R, BF16, E4, E5, I8, mybir,
                )

    nc.compile()
    return nc


def _batches(
    nc, tc, x8_sbs, xr8_sbs, xb_sbs, qkp, vp, ep, scrp, rp, op, smallp, vsp,
    pz, wqk_sb, wv_sb, wo_sb, b_qk, bv2_sb, ebias, ident, out_d,
    F32, F32R, BF16, E4, E5, I8, mybir,
):
    """Flat (batch, head) stage pipeline: stage s emits scores+exp for its own
    head and the AV/normalization for stage s-1, so the ACT engine (the
    bottleneck: 64 exp passes) never waits on the softmax->AV->resT chain."""
    DR = mybir.MatmulPerfMode.DoubleRow
    EXP = mybir.ActivationFunctionType.Exp
    IDENT = mybir.ActivationFunctionType.Identity
    MULT = mybir.AluOpType.mult
    ADD = mybir.AluOpType.add

    bctx: dict = {}

    def get_bctx(b):
        if b not in bctx:
            qk_sb = qkp.tile([128, 2 * NH, S], BF16, name=f"qk_{b}", tag="qk")
            v_sb = vp.tile([128, ST // 2, 2 * F], BF16, name=f"v_{b}", tag="v")
            resT_sb = rp.tile([128, NH, S], E4, name=f"resT_{b}", tag="resT")
            bctx[b] = (qk_sb, v_sb, resT_sb)
        return bctx[b]

    def qk_proj(b, t):
        # Q^T/K^T f-tile t: qk[:, t, s] = w_qk[:, t].T @ (x8 + xr8)
        qk_sb = get_bctx(b)[0]
        acc = pz.tile([128, S], F32, name="qka", tag="wp", bufs=1)
        srcs = (x8_sbs[b], xr8_sbs[b]) if USE_FEEDBACK else (x8_sbs[b],)
        for kp in range(KT // 2):
            for si, src in enumerate(srcs):
                for n in range(NT):
                    nc.tensor.matmul(
                        acc[:, bass_ts(n, 512)],
                        wqk_sb[:, 2 * kp : 2 * kp + 2, bass_ts(t, 128)],
                        src[:, 2 * kp : 2 * kp + 2, bass_ts(n, 512)],
                        start=(kp == 0 and si == 0),
                        stop=(kp == KT // 2 - 1 and si == len(srcs) - 1),
                        perf_mode=DR,
                    )
        if t in ACT_CONV_TILES:
            nc.scalar.activation(
                out=qk_sb[:, t, :], in_=acc, func=IDENT, bias=b_qk[:, t : t + 1]
            )
        else:
            nc.vector.tensor_scalar_add(qk_sb[:, t, :], acc, b_qk[:, t : t + 1])

    def v_proj(b, u):
        # V rows for st-pair u: v_sb[:, u, i*F + f] = V[j=128*(2u+i)+p, f]
        v_sb = get_bctx(b)[1]
        acc = pz.tile([128, S], F32, name="vacc", tag="wp", bufs=1)
        for i in range(2):
            st = 2 * u + i
            for kp in range(KT // 2):
                nc.tensor.matmul(
                    acc[:, bass_ts(i, 512)],
                    x8_sbs[b][:, 2 * kp : 2 * kp + 2, bass_ts(st, 128)],
                    wv_sb[:, 2 * kp : 2 * kp + 2, :],
                    start=(kp == 0),
                    stop=(kp == KT // 2 - 1),
                    perf_mode=DR,
                )
        nc.vector.tensor_tensor(v_sb[:, u, :], acc, bv2_sb, op=ADD)

    def out_proj(b, ct):
        # psum = 2^K*xb + wo8.T @ resT; out = psum * 2^-K
        resT_sb = get_bctx(b)[2]
        out_t = op.tile([128, S], F32, name="out_t", tag="out")
        acc = pz.tile([128, S], F32, name="oacc", tag="wp", bufs=1)
        for n in range(NT):
            nc.tensor.matmul(
                acc[:, bass_ts(n, 512)],
                ident,
                xb_sbs[b][:, ct, bass_ts(n, 512)],
                start=True,
                stop=False,
            )
            for kp in range(KT // 2):
                nc.tensor.matmul(
                    acc[:, bass_ts(n, 512)],
                    wo_sb[:, 2 * kp : 2 * kp + 2, bass_ts(ct, 128)],
                    resT_sb[:, 2 * kp : 2 * kp + 2, bass_ts(n, 512)],
                    start=False,
                    stop=(kp == KT // 2 - 1),
                    perf_mode=DR,
                )
        if ct in OUT_CONV_ACT:
            nc.scalar.mul(out_t, acc, float(2.0 ** (-K_RES)))
        else:
            nc.vector.tensor_scalar(out_t, acc, float(2.0 ** (-K_RES)), None, MULT)
        nc.sync.dma_start(
            out=out_d[b, :, :].rearrange("(k p) s -> p k s", p=128)[:, ct, :],
            in_=out_t,
        )

    # ---- static filler schedule: PE-side proj/out work interleaved at the
    # u-granularity of each stage so the wide-psum ring never stalls PE.
    NS = BL * NH
    fillers = {s: [[] for _ in range(4)] for s in range(NS)}
    epilogue = []
    for b in range(BL):
        s0 = b * NH
        if b > 0:
            fillers[s0 - 1][2].append(lambda b=b: qk_proj(b, 0))
            fillers[s0 - 1][3].append(lambda b=b: qk_proj(b, 1))
        for u in range(4):
            fillers[s0][u].append(lambda b=b, u=u: v_proj(b, u))
        fillers[s0][1].append(lambda b=b: qk_proj(b, 2))
        fillers[s0][2].append(lambda b=b: qk_proj(b, 3))
        for h in (1, 2):
            fillers[s0 + h][1].append(lambda b=b, h=h: qk_proj(b, 2 * h + 2))
            fillers[s0 + h][3].append(lambda b=b, h=h: qk_proj(b, 2 * h + 3))
        for ct in range(KT):
            tgt = s0 + NH + 1 + ct // 2
            if tgt < NS:
                fillers[tgt][(ct % 2) * 2].append(lambda b=b, ct=ct: out_proj(b, ct))
            else:
                epilogue.append(lambda b=b, ct=ct: out_proj(b, ct))

    def av_stage(st, u_pair):
        # normalization + AV for stage st, pairs u_pair (emitted from the
        # following stage so it overlaps that stage's exps)
        b, h, e8s, ssum, racc = st
        sch = h in SCH_HEADS
        v_sb = get_bctx(b)[1]
        for u in u_pair:
            v_sc = vsp.tile([128, 2, DK], E4, name="vsc", tag="vsc")
            vsc_eng = nc.gpsimd if VSC_POOL else nc.vector
            for i in range(2):
                jt = 2 * u + i
                vsc_eng.tensor_scalar(
                    v_sc[:, i, :],
                    v_sb[:, u, bass_ts(i, 512)][:, bass_ts(h, DK)],
                    ssum[:, ST + jt : ST + jt + 1], float(2.0 ** P_VSC),
                    MULT, MULT,
                )
            e8 = e8s[u]
            for n in range(NT):
                rhs = e8[:, :, bass_ts(n, 512)]
                nc.tensor.matmul(
                    racc[:, bass_ts(n, 512)],
                    v_sc,
                    rhs.bitcast(E5) if sch else rhs,
                    start=(u == 0),
                    stop=(u == ST // 2 - 1),
                    perf_mode=DR,
                )

    def res_stage(st):
        b, h, e8s, ssum, racc = st
        resT_sb = get_bctx(b)[2]
        if h in RES_CONV_ACT:
            nc.scalar.mul(resT_sb[:, h, :], racc, float(2.0 ** (K_RES - P_VSC)))
        else:
            nc.vector.tensor_scalar(
                resT_sb[:, h, :], racc, float(2.0 ** (K_RES - P_VSC)), None, MULT
            )

    qk_proj(0, 0)
    qk_proj(0, 1)

    prev = None
    for s in range(NS):
        b, h = divmod(s, NH)
        qk_sb = get_bctx(b)[0]
        sch = h in SCH_HEADS
        ssum = smallp.tile([128, 2 * ST], F32, name="ssum", tag="ssum")
        if NO_ACCUM_PROBE or NO_EXP_PROBE:
            nc.vector.memset(ssum[:, 0:ST], 1.0)
        e8s = []
        for u in range(ST // 2):
            e8 = ep.tile([128, 2, S], E4, name=f"e8_{u}", tag="e8")
            e8s.append(e8)
            for i in range(2):
                jt = 2 * u + i
                sacc = pz.tile([128, S], F32, name="sacc", tag="sacc", bufs=2)
                for n in range(NT):
                    nc.tensor.matmul(
                        sacc[:, bass_ts(n, 512)],
                        qk_sb[:, 2 * h + 1, bass_ts(jt, 128)],
                        qk_sb[:, 2 * h, bass_ts(n, 512)],
                        start=True,
                        stop=True,
                    )
                if sch:
                    nc.vector.tensor_scalar(
                        e8[:, i, :].bitcast(I8), sacc, SCH_A, SCH_B, MULT, ADD
                    )
                    scr = scrp.tile([128, S], E5, name="scr", tag="scr")
                    nc.vector.tensor_scalar(
                        scr, e8[:, i, :].bitcast(E5), 1.0, None, MULT, ADD,
                        accum_out=ssum[:, jt : jt + 1],
                    )
                elif NO_EXP_PROBE:
                    pass
                elif NO_ACCUM_PROBE:
                    nc.scalar.activation(
                        out=e8[:, i, :], in_=sacc, func=EXP, scale=SCALE,
                        bias=ebias[:, 0:1],
                    )
                else:
                    nc.scalar.activation(
                        out=e8[:, i, :], in_=sacc, func=EXP, scale=SCALE,
                        bias=ebias[:, 0:1], accum_out=ssum[:, jt : jt + 1],
                    )
            for f in fillers[s][u]:
                f()
            if prev is not None:
                if u == 1:
                    av_stage(prev, (0, 1))
                elif u == 3:
                    av_stage(prev, (2, 3))
        if prev is not None:
            res_stage(prev)
        # normalizer for this stage (consumed by av_stage from stage s+1)
        nc.vector.reciprocal(ssum[:, ST : 2 * ST], ssum[:, 0:ST])
        racc = pz.tile([128, S], F32, name="racc", tag="racc", bufs=1)
        prev = (b, h, e8s, ssum, racc)

    av_stage(prev, (0, 1))
    av_stage(prev, (2, 3))
    res_stage(prev)
    for f in epilogue:
        f()


def bass_ts(i, size):
    import concourse.bass as bass

    return bass.ts(i, size)


def _prep_inputs(x, w_proj, b_proj, w_out, b_out):
    """Host-side quantization + reshaping into the layouts the kernel expects."""
    E4 = ml_dtypes.float8_e4m3
    x_f = np.ascontiguousarray(np.asarray(x, np.float32).reshape(B, C, S))
    b_out = np.asarray(b_out, np.float32)
    xb = np.ascontiguousarray(x_f + b_out[None, :, None])
    x8 = x_f.astype(E4)
    xr8 = np.ascontiguousarray(x_f - x8.astype(np.float32)).astype(E4)
    x8 = np.ascontiguousarray(x8)

    wT = np.asarray(w_proj, np.float32).T  # [C, 3F]
    w_qkT = np.concatenate(
        [wT[:, h * 384 : h * 384 + 256] for h in range(NH)], axis=1
    )  # [C, 2F]; col tile t=2h -> q_h, t=2h+1 -> k_h
    w_vT = np.concatenate(
        [wT[:, h * 384 + 256 : h * 384 + 384] for h in range(NH)], axis=1
    )  # [C, F]
    w_outT = np.asarray(w_out, np.float32).T  # [F, C]
    wqk8 = np.ascontiguousarray(w_qkT).astype(E4)
    wv8 = np.ascontiguousarray(w_vT).astype(E4)
    wo8 = np.ascontiguousarray(w_outT).astype(E4)

    b_proj = np.asarray(b_proj, np.float32)
    b_qk = np.stack(
        [
            b_proj[h * 384 + half * 128 : h * 384 + half * 128 + 128]
            for h in range(NH)
            for half in range(2)
        ],
        axis=1,
    )  # [128, 2*NH]
    b_v = np.concatenate([b_proj[h * 384 + 256 : h * 384 + 384] for h in range(NH)])
    bv2 = np.broadcast_to(np.concatenate([b_v, b_v]), (128, 2 * F))
    eb = np.full((128, 1), -M_EXP * np.log(2.0), np.float32)
    ident = np.eye(128, dtype=np.float32) * float(2.0 ** K_RES)
    consts = np.ascontiguousarray(
        np.concatenate([b_qk, bv2, eb, ident], axis=1), dtype=np.float32
    )
    return x8, xr8, xb, wqk8, wv8, wo8, consts


def kernel(x, w_proj, b_proj, w_out, b_out, n_heads):
    from concourse.bass_utils import run_bass_kernel_spmd

    assert int(n_heads) == NH
    x8, xr8, xb, wqk8, wv8, wo8, consts = _prep_inputs(x, w_proj, b_proj, w_out, b_out)

    if "nc" not in _CACHE:
        _CACHE["nc"] = _build()
    nc = _CACHE["nc"]

    in_maps = [
        {
            "x8": np.ascontiguousarray(x8[c * BL : (c + 1) * BL]),
            "xr8": np.ascontiguousarray(xr8[c * BL : (c + 1) * BL]),
            "xb": np.ascontiguousarray(xb[c * BL : (c + 1) * BL]),
            "wqk8": wqk8,
            "wv8": wv8,
            "wo8": wo8,
            "consts": consts,
        }
        for c in range(NCORES)
    ]
    res = run_bass_kernel_spmd(nc, in_maps, list(range(NCORES)))
    out = np.concatenate([res.results[c]["out"] for c in range(NCORES)], axis=0)
    return out.reshape(B, C, 32, 32)


# revision 15
# speedup vs baseline: 1.2980x; 1.0537x over previous
"""AttentionBlock Trainium2 kernel (fp8 DoubleRow + multi-engine softmax).

Reference computation (B=16, C=512, H=W=32, n_heads=4, d_k=128):
    xs   = x.reshape(B,C,S).T            # [B, S, C],  S = 1024
    qkv  = xs @ w_proj.T + b_proj        # [B, S, 1536]
    S_   = einsum('bihd,bjhd->bijh', q, k) * d_k**-0.5
    attn = softmax(S_, axis=1)           # over the QUERY axis i (source quirk)
    res  = einsum('bijh,bjhd->bihd', attn, v)
    out  = res @ w_out.T + b_out + xs    # residual
    return out.T.reshape(B, C, H, W)

Strategy: data-parallel over batch, 2 batches per core on 8 cores.

Dtype plan (validated vs the reference in numpy, rel err ~9.4e-3 < 2e-2):
  - QKV projections: e4m3 x & weights, DoubleRow fp8 matmuls (2 k-tiles of 128
    contracted per MM, 2 output cols/cycle). QK additionally accumulates an
    x-residual correction term (xr8 = e4m3(x - e4m3(x))) to halve proj noise.
  - Q,K stored bf16; scores matmul bf16 (contraction d_k=128, no DR possible).
  - exp on ACT: scale=dk**-0.5, bias=-2*ln2 keeps e4m3 in range; accum_out
    produces the softmax-axis sums. (Optionally SCH_HEADS run exp on DVE via a
    Schraudolph bit-trick in e5m2; measured slower on HW - off by default.)
  - AV: DoubleRow fp8, stationary v_sc = e4m3(V * 2^P / Z_j) (Pool engine),
    moving e8 tiles.
  - out-proj: DoubleRow e4m3 (resT stored e4m3 scaled 2^(K-P)); the residual
    (xb = x + b_out, fp32) is injected into the same psum group by a leading
    identity matmul (2^K*I, fp32r) so the final psum->sbuf pass is a single
    multiply by 2^-K.

All psum->sbuf evacuation passes use [128,1024] (2-bank) tiles to halve the
per-instruction overhead on DVE/ACT, which HW timing showed dominates.
"""
import sys

for _p in (
    "/opt/trn_rl_repo",
    "/root/.axon_site",
    "/root/.axon_site/_ro/trn_rl_repo",
    "/root/.axon_site/_ro/pypackages",
):
    if _p not in sys.path:
        sys.path.append(_p)

import numpy as np
import ml_dtypes

B = 16
C = 512
S = 1024  # H*W
NH = 4
DK = 128
F = NH * DK  # 512
NCORES = 8
BL = B // NCORES  # batches per core
KT = C // 128  # 4 contraction tiles over channels
ST = S // 128  # 8 seq tiles
NT = S // 512  # 2 free-dim chunks of 512
SCALE = float(DK) ** -0.5

M_EXP = 2          # exp computed as exp(t - M_EXP*ln2); 2^M_EXP folded into Z
P_VSC = 7          # v_sc = V * 2^P_VSC / Z
K_RES = 6          # resT scaled 2^(K_RES-P_VSC); residual via 2^K_RES*I matmul
SCH_HEADS = ()     # unused (see SCH_PAIRS)
SCH_PAIRS = ()     # (h, u) jt-pairs whose exp runs on DVE (Schraudolph e5m2)
ACT_CONV_TILES = ()  # qk f-tiles whose psum->bf16 conversion runs on ACT
OUT_CONV_ACT = ()  # out-proj c-tiles whose psum->f32 conversion runs on ACT
RES_CONV_ACT = ()  # heads whose racc->resT conversion runs on ACT
VSC_POOL = True    # v_sc scaling on Pool engine (False: DVE)
USE_FEEDBACK = False  # QK proj xr8 correction: off (PE pacing gates ACT; -9us)
NO_ACCUM_PROBE = False  # TIMING PROBE ONLY: drop accum_out (wrong results)
NO_EXP_PROBE = False    # TIMING PROBE ONLY: skip exp entirely (wrong results)
SCH_A = float(4.0 / np.log(2.0) * SCALE)
SCH_B = float(4.0 * (15 - M_EXP) - 0.45 + 0.5)  # +0.5: DVE int8 convert truncates

_CACHE: dict = {}


def _build(repeat=1):
    """Build the kernel. repeat>1 wraps the per-call workload in an on-device
    For_i loop (timing only; amortizes the ~10ms axon dispatch)."""
    import contextlib

    import concourse.tile as tile
    from concourse import bacc, mybir

    F32 = mybir.dt.float32
    F32R = mybir.dt.float32r
    BF16 = mybir.dt.bfloat16
    E4 = mybir.dt.float8e4
    E5 = mybir.dt.float8e5
    I8 = mybir.dt.int8

    nc = bacc.Bacc("TRN2", debug=False)
    x8_d = nc.dram_tensor("x8", [BL, C, S], E4, kind="ExternalInput").ap()
    xr8_d = nc.dram_tensor("xr8", [BL, C, S], E4, kind="ExternalInput").ap()
    xb_d = nc.dram_tensor("xb", [BL, C, S], F32, kind="ExternalInput").ap()
    xb_r = xb_d.bitcast(F32R)
    wqk_d = nc.dram_tensor("wqk8", [C, 2 * F], E4, kind="ExternalInput").ap()
    wv_d = nc.dram_tensor("wv8", [C, F], E4, kind="ExternalInput").ap()
    wo_d = nc.dram_tensor("wo8", [F, C], E4, kind="ExternalInput").ap()
    NCONST = 2 * NH + 2 * F + 1 + 128
    consts_d = nc.dram_tensor(
        "consts", [128, NCONST], F32, kind="ExternalInput"
    ).ap().bitcast(F32R)
    out_d = nc.dram_tensor("out", [BL, C, S], F32, kind="ExternalOutput").ap()

    wqk_r = wqk_d.rearrange("(k p) m -> p k m", p=128)
    wv_r = wv_d.rearrange("(k p) m -> p k m", p=128)
    wo_r = wo_d.rearrange("(k p) m -> p k m", p=128)

    with tile.TileContext(nc) as tc:
        with (
            tc.tile_pool(name="const", bufs=1) as constp,
            tc.tile_pool(name="xp", bufs=2) as xp,
            tc.tile_pool(name="qkp", bufs=2) as qkp,
            tc.tile_pool(name="vp", bufs=2) as vp,
            tc.tile_pool(name="ep", bufs=6) as ep,
            tc.tile_pool(name="scr", bufs=2) as scrp,
            tc.tile_pool(name="rp", bufs=2) as rp,
            tc.tile_pool(name="op", bufs=2) as op,
            tc.tile_pool(name="small", bufs=8) as smallp,
            tc.tile_pool(name="vs", bufs=8) as vsp,
            # psum: one pool, per-tag rings: "wp" (proj/out-proj) 1x[128,1024],
            # "sacc" 2x[128,1024], "racc" 1x[128,1024] -> 2+4+2 = 8 banks.
            tc.tile_pool(name="pz", bufs=1, space="PSUM") as pz,
        ):
            wqk_sb = constp.tile([128, KT, 2 * F], E4)
            wv_sb = constp.tile([128, KT, F], E4)
            wo_sb = constp.tile([128, KT, C], E4)
            consts_sb = constp.tile([128, NCONST], F32R)
            x8_sbs = [xp.tile([128, KT, S], E4, name=f"x8_{b}", tag="x8") for b in range(BL)]
            xr8_sbs = [xp.tile([128, KT, S], E4, name=f"xr8_{b}", tag="xr8") for b in range(BL)]
            xb_sbs = [xp.tile([128, KT, S], F32R, name=f"xb_{b}", tag="xb") for b in range(BL)]

            # DMA order: first batch's x8 + qk weights first (gate the first
            # matmuls), then the rest. One DMA per (tensor, batch) via 3D APs.
            def xre(ap_d, b):
                return ap_d[b].rearrange("(k p) s -> p k s", p=128)

            nc.sync.dma_start(out=x8_sbs[0], in_=xre(x8_d, 0))
            nc.sync.dma_start(out=wqk_sb, in_=wqk_r)
            nc.sync.dma_start(out=consts_sb, in_=consts_d)
            nc.sync.dma_start(out=xr8_sbs[0], in_=xre(xr8_d, 0))
            nc.sync.dma_start(out=wv_sb, in_=wv_r)
            for b in range(1, BL):
                nc.sync.dma_start(out=x8_sbs[b], in_=xre(x8_d, b))
                nc.sync.dma_start(out=xr8_sbs[b], in_=xre(xr8_d, b))
            nc.sync.dma_start(out=wo_sb, in_=wo_r)
            for b in range(BL):
                nc.sync.dma_start(out=xb_sbs[b], in_=xre(xb_r, b))

            b_qk = consts_sb[:, 0 : 2 * NH].bitcast(F32)
            bv2_sb = consts_sb[:, 2 * NH : 2 * NH + 2 * F].bitcast(F32)
            ebias = consts_sb[:, 2 * NH + 2 * F : 2 * NH + 2 * F + 1].bitcast(F32)
            ident = consts_sb[:, 2 * NH + 2 * F + 1 :]  # [128,128] = 2^K_RES*I (fp32r)

            rep_ctx = tc.For_i(0, repeat, 1) if repeat > 1 else contextlib.nullcontext()
            post_loop = []
            with rep_ctx:
                _batches(
                    nc, tc, x8_sbs, xr8_sbs, xb_sbs, qkp, vp, ep, scrp, rp, op,
                    smallp, vsp, pz, wqk_sb, wv_sb, wo_sb, b_qk, bv2_sb,
                    ebias, ident, out_d, F32, F32R, BF16, E4, E5, I8, mybir,
                    defer_tail=(repeat > 1), post_loop=post_loop,
                )
            for f in post_loop:
                f()

    nc.compile()
    return nc


def _batches(
    nc, tc, x8_sbs, xr8_sbs, xb_sbs, qkp, vp, ep, scrp, rp, op, smallp, vsp,
    pz, wqk_sb, wv_sb, wo_sb, b_qk, bv2_sb, ebias, ident, out_d,
    F32, F32R, BF16, E4, E5, I8, mybir, defer_tail=False, post_loop=None,
):
    """Flat (batch, head) stage pipeline: stage s emits scores+exp for its own
    head and the AV/normalization for stage s-1, so the ACT engine (the
    bottleneck: 64 exp passes) never waits on the softmax->AV->resT chain."""
    DR = mybir.MatmulPerfMode.DoubleRow
    EXP = mybir.ActivationFunctionType.Exp
    IDENT = mybir.ActivationFunctionType.Identity
    MULT = mybir.AluOpType.mult
    ADD = mybir.AluOpType.add

    bctx: dict = {}

    def get_bctx(b):
        if b not in bctx:
            qk_sb = qkp.tile([128, 2 * NH, S], BF16, name=f"qk_{b}", tag="qk")
            v_sb = vp.tile([128, ST // 2, 2 * F], BF16, name=f"v_{b}", tag="v")
            resT_sb = rp.tile([128, NH, S], E4, name=f"resT_{b}", tag="resT")
            bctx[b] = (qk_sb, v_sb, resT_sb)
        return bctx[b]

    def qk_proj(b, t):
        # Q^T/K^T f-tile t: qk[:, t, s] = w_qk[:, t].T @ (x8 + xr8)
        qk_sb = get_bctx(b)[0]
        acc = pz.tile([128, S], F32, name="qka", tag="wp", bufs=1)
        srcs = (x8_sbs[b], xr8_sbs[b]) if USE_FEEDBACK else (x8_sbs[b],)
        for kp in range(KT // 2):
            for si, src in enumerate(srcs):
                for n in range(NT):
                    nc.tensor.matmul(
                        acc[:, bass_ts(n, 512)],
                        wqk_sb[:, 2 * kp : 2 * kp + 2, bass_ts(t, 128)],
                        src[:, 2 * kp : 2 * kp + 2, bass_ts(n, 512)],
                        start=(kp == 0 and si == 0),
                        stop=(kp == KT // 2 - 1 and si == len(srcs) - 1),
                        perf_mode=DR,
                    )
        if t in ACT_CONV_TILES:
            nc.scalar.activation(
                out=qk_sb[:, t, :], in_=acc, func=IDENT, bias=b_qk[:, t : t + 1]
            )
        else:
            nc.vector.tensor_scalar_add(qk_sb[:, t, :], acc, b_qk[:, t : t + 1])

    def v_proj(b, u):
        # V rows for st-pair u: v_sb[:, u, i*F + f] = V[j=128*(2u+i)+p, f]
        v_sb = get_bctx(b)[1]
        acc = pz.tile([128, S], F32, name="vacc", tag="wp", bufs=1)
        for i in range(2):
            st = 2 * u + i
            for kp in range(KT // 2):
                nc.tensor.matmul(
                    acc[:, bass_ts(i, 512)],
                    x8_sbs[b][:, 2 * kp : 2 * kp + 2, bass_ts(st, 128)],
                    wv_sb[:, 2 * kp : 2 * kp + 2, :],
                    start=(kp == 0),
                    stop=(kp == KT // 2 - 1),
                    perf_mode=DR,
                )
        nc.vector.tensor_tensor(v_sb[:, u, :], acc, bv2_sb, op=ADD)

    def out_proj(b, ct):
        # psum = 2^K*xb + wo8.T @ resT; out = psum * 2^-K
        resT_sb = get_bctx(b)[2]
        out_t = op.tile([128, S], F32, name="out_t", tag="out")
        acc = pz.tile([128, S], F32, name="oacc", tag="wp", bufs=1)
        for n in range(NT):
            nc.tensor.matmul(
                acc[:, bass_ts(n, 512)],
                ident,
                xb_sbs[b][:, ct, bass_ts(n, 512)],
                start=True,
                stop=False,
            )
            for kp in range(KT // 2):
                nc.tensor.matmul(
                    acc[:, bass_ts(n, 512)],
                    wo_sb[:, 2 * kp : 2 * kp + 2, bass_ts(ct, 128)],
                    resT_sb[:, 2 * kp : 2 * kp + 2, bass_ts(n, 512)],
                    start=False,
                    stop=(kp == KT // 2 - 1),
                    perf_mode=DR,
                )
        if ct in OUT_CONV_ACT:
            nc.scalar.mul(out_t, acc, float(2.0 ** (-K_RES)))
        else:
            nc.vector.tensor_scalar(out_t, acc, float(2.0 ** (-K_RES)), None, MULT)
        nc.sync.dma_start(
            out=out_d[b, :, :].rearrange("(k p) s -> p k s", p=128)[:, ct, :],
            in_=out_t,
        )

    # ---- static filler schedule: PE-side proj/out work interleaved at the
    # u-granularity of each stage so the wide-psum ring never stalls PE.
    NS = BL * NH
    fillers = {s: [[] for _ in range(4)] for s in range(NS)}
    epilogue = []
    for b in range(BL):
        s0 = b * NH
        if b > 0:
            fillers[s0 - 1][2].append(lambda b=b: qk_proj(b, 0))
            fillers[s0 - 1][3].append(lambda b=b: qk_proj(b, 1))
        elif defer_tail:
            # rotate the next iteration's first projections into the last
            # stage (their own iteration's readers are long done; idempotent)
            fillers[NS - 1][1].append(lambda: qk_proj(0, 0))
            fillers[NS - 1][3].append(lambda: qk_proj(0, 1))
        for u in range(4):
            fillers[s0][u].append(lambda b=b, u=u: v_proj(b, u))
        fillers[s0][1].append(lambda b=b: qk_proj(b, 2))
        fillers[s0][2].append(lambda b=b: qk_proj(b, 3))
        for h in (1, 2):
            fillers[s0 + h][1].append(lambda b=b, h=h: qk_proj(b, 2 * h + 2))
            fillers[s0 + h][3].append(lambda b=b, h=h: qk_proj(b, 2 * h + 3))
        for ct in range(KT):
            tgt = s0 + NH + 1 + ct // 2
            if tgt < NS:
                fillers[tgt][(ct % 2) * 2].append(lambda b=b, ct=ct: out_proj(b, ct))
            elif defer_tail:
                # emit into the NEXT loop iteration's early stages (reads the
                # previous iteration's resT ring slot); the post-loop drain
                # below produces the final correct output.
                fillers[tgt - NS][(ct % 2) * 2 + 1].append(
                    lambda b=b, ct=ct: out_proj(b, ct)
                )
                post_loop.append(lambda b=b, ct=ct: out_proj(b, ct))
            else:
                epilogue.append(lambda b=b, ct=ct: out_proj(b, ct))

    def av_stage(st, u_pair):
        # normalization + AV for stage st, pairs u_pair (emitted from the
        # following stage so it overlaps that stage's exps)
        b, h, e8s, ssum, racc = st
        v_sb = get_bctx(b)[1]
        for u in u_pair:
            sch = (h, u) in SCH_PAIRS
            v_sc = vsp.tile([128, 2, DK], E4, name="vsc", tag="vsc")
            vsc_eng = nc.gpsimd if VSC_POOL else nc.vector
            for i in range(2):
                jt = 2 * u + i
                vsc_eng.tensor_scalar(
                    v_sc[:, i, :],
                    v_sb[:, u, bass_ts(i, 512)][:, bass_ts(h, DK)],
                    ssum[:, ST + jt : ST + jt + 1], float(2.0 ** P_VSC),
                    MULT, MULT,
                )
            e8 = e8s[u]
            for n in range(NT):
                rhs = e8[:, :, bass_ts(n, 512)]
                nc.tensor.matmul(
                    racc[:, bass_ts(n, 512)],
                    v_sc,
                    rhs.bitcast(E5) if sch else rhs,
                    start=(u == 0),
                    stop=(u == ST // 2 - 1),
                    perf_mode=DR,
                )

    def res_stage(st):
        b, h, e8s, ssum, racc = st
        resT_sb = get_bctx(b)[2]
        if h in RES_CONV_ACT:
            nc.scalar.mul(resT_sb[:, h, :], racc, float(2.0 ** (K_RES - P_VSC)))
        else:
            nc.vector.tensor_scalar(
                resT_sb[:, h, :], racc, float(2.0 ** (K_RES - P_VSC)), None, MULT
            )

    qk_proj(0, 0)
    qk_proj(0, 1)

    prev = None
    for s in range(NS):
        b, h = divmod(s, NH)
        qk_sb = get_bctx(b)[0]
        ssum = smallp.tile([128, 2 * ST], F32, name="ssum", tag="ssum")
        if NO_ACCUM_PROBE or NO_EXP_PROBE:
            nc.vector.memset(ssum[:, 0:ST], 1.0)
        e8s = []
        for u in range(ST // 2):
            sch = (h, u) in SCH_PAIRS
            e8 = ep.tile([128, 2, S], E4, name=f"e8_{u}", tag="e8")
            e8s.append(e8)
            for i in range(2):
                jt = 2 * u + i
                sacc = pz.tile([128, S], F32, name="sacc", tag="sacc", bufs=2)
                for n in range(NT):
                    nc.tensor.matmul(
                        sacc[:, bass_ts(n, 512)],
                        qk_sb[:, 2 * h + 1, bass_ts(jt, 128)],
                        qk_sb[:, 2 * h, bass_ts(n, 512)],
                        start=True,
                        stop=True,
                    )
                if sch:
                    nc.vector.tensor_scalar(
                        e8[:, i, :].bitcast(I8), sacc, SCH_A, SCH_B, MULT, ADD
                    )
                    scr = scrp.tile([128, S], E5, name="scr", tag="scr")
                    nc.vector.tensor_scalar(
                        scr, e8[:, i, :].bitcast(E5), 1.0, None, MULT, ADD,
                        accum_out=ssum[:, jt : jt + 1],
                    )
                elif NO_EXP_PROBE:
                    pass
                elif NO_ACCUM_PROBE:
                    nc.scalar.activation(
                        out=e8[:, i, :], in_=sacc, func=EXP, scale=SCALE,
                        bias=ebias[:, 0:1],
                    )
                else:
                    nc.scalar.activation(
                        out=e8[:, i, :], in_=sacc, func=EXP, scale=SCALE,
                        bias=ebias[:, 0:1], accum_out=ssum[:, jt : jt + 1],
                    )
            for f in fillers[s][u]:
                f()
            if prev is not None:
                if u == 1:
                    av_stage(prev, (0, 1))
                elif u == 3:
                    av_stage(prev, (2, 3))
        if prev is not None:
            res_stage(prev)
        # normalizer for this stage (consumed by av_stage from stage s+1)
        nc.vector.reciprocal(ssum[:, ST : 2 * ST], ssum[:, 0:ST])
        racc = pz.tile([128, S], F32, name="racc", tag="racc", bufs=1)
        prev = (b, h, e8s, ssum, racc)

    av_stage(prev, (0, 1))
    av_stage(prev, (2, 3))
    res_stage(prev)
    for f in epilogue:
        f()


def bass_ts(i, size):
    import concourse.bass as bass

    return bass.ts(i, size)


def _prep_inputs(x, w_proj, b_proj, w_out, b_out):
    """Host-side quantization + reshaping into the layouts the kernel expects."""
    E4 = ml_dtypes.float8_e4m3
    x_f = np.ascontiguousarray(np.asarray(x, np.float32).reshape(B, C, S))
    b_out = np.asarray(b_out, np.float32)
    xb = np.ascontiguousarray(x_f + b_out[None, :, None])
    x8 = x_f.astype(E4)
    xr8 = np.ascontiguousarray(x_f - x8.astype(np.float32)).astype(E4)
    x8 = np.ascontiguousarray(x8)

    wT = np.asarray(w_proj, np.float32).T  # [C, 3F]
    w_qkT = np.concatenate(
        [wT[:, h * 384 : h * 384 + 256] for h in range(NH)], axis=1
    )  # [C, 2F]; col tile t=2h -> q_h, t=2h+1 -> k_h
    w_vT = np.concatenate(
        [wT[:, h * 384 + 256 : h * 384 + 384] for h in range(NH)], axis=1
    )  # [C, F]
    w_outT = np.asarray(w_out, np.float32).T  # [F, C]
    wqk8 = np.ascontiguousarray(w_qkT).astype(E4)
    wv8 = np.ascontiguousarray(w_vT).astype(E4)
    wo8 = np.ascontiguousarray(w_outT).astype(E4)

    b_proj = np.asarray(b_proj, np.float32)
    b_qk = np.stack(
        [
            b_proj[h * 384 + half * 128 : h * 384 + half * 128 + 128]
            for h in range(NH)
            for half in range(2)
        ],
        axis=1,
    )  # [128, 2*NH]
    b_v = np.concatenate([b_proj[h * 384 + 256 : h * 384 + 384] for h in range(NH)])
    bv2 = np.broadcast_to(np.concatenate([b_v, b_v]), (128, 2 * F))
    eb = np.full((128, 1), -M_EXP * np.log(2.0), np.float32)
    ident = np.eye(128, dtype=np.float32) * float(2.0 ** K_RES)
    consts = np.ascontiguousarray(
        np.concatenate([b_qk, bv2, eb, ident], axis=1), dtype=np.float32
    )
    return x8, xr8, xb, wqk8, wv8, wo8, consts


def kernel(x, w_proj, b_proj, w_out, b_out, n_heads):
    from concourse.bass_utils import run_bass_kernel_spmd

    assert int(n_heads) == NH
    x8, xr8, xb, wqk8, wv8, wo8, consts = _prep_inputs(x, w_proj, b_proj, w_out, b_out)

    if "nc" not in _CACHE:
        _CACHE["nc"] = _build()
    nc = _CACHE["nc"]

    in_maps = [
        {
            "x8": np.ascontiguousarray(x8[c * BL : (c + 1) * BL]),
            "xr8": np.ascontiguousarray(xr8[c * BL : (c + 1) * BL]),
            "xb": np.ascontiguousarray(xb[c * BL : (c + 1) * BL]),
            "wqk8": wqk8,
            "wv8": wv8,
            "wo8": wo8,
            "consts": consts,
        }
        for c in range(NCORES)
    ]
    res = run_bass_kernel_spmd(nc, in_maps, list(range(NCORES)))
    out = np.concatenate([res.results[c]["out"] for c in range(NCORES)], axis=0)
    return out.reshape(B, C, 32, 32)


# revision 16
# speedup vs baseline: 1.3089x; 1.0084x over previous
"""AttentionBlock Trainium2 kernel (fp8 DoubleRow + multi-engine softmax).

Reference computation (B=16, C=512, H=W=32, n_heads=4, d_k=128):
    xs   = x.reshape(B,C,S).T            # [B, S, C],  S = 1024
    qkv  = xs @ w_proj.T + b_proj        # [B, S, 1536]
    S_   = einsum('bihd,bjhd->bijh', q, k) * d_k**-0.5
    attn = softmax(S_, axis=1)           # over the QUERY axis i (source quirk)
    res  = einsum('bijh,bjhd->bihd', attn, v)
    out  = res @ w_out.T + b_out + xs    # residual
    return out.T.reshape(B, C, H, W)

Strategy: data-parallel over batch, 2 batches per core on 8 cores.

Dtype plan (validated vs the reference in numpy, rel err ~9.4e-3 < 2e-2):
  - QKV projections: e4m3 x & weights, DoubleRow fp8 matmuls (2 k-tiles of 128
    contracted per MM, 2 output cols/cycle). QK additionally accumulates an
    x-residual correction term (xr8 = e4m3(x - e4m3(x))) to halve proj noise.
  - Q,K stored bf16; scores matmul bf16 (contraction d_k=128, no DR possible).
  - exp on ACT: scale=dk**-0.5, bias=-2*ln2 keeps e4m3 in range; accum_out
    produces the softmax-axis sums. (Optionally SCH_HEADS run exp on DVE via a
    Schraudolph bit-trick in e5m2; measured slower on HW - off by default.)
  - AV: DoubleRow fp8, stationary v_sc = e4m3(V * 2^P / Z_j) (Pool engine),
    moving e8 tiles.
  - out-proj: DoubleRow e4m3 (resT stored e4m3 scaled 2^(K-P)); the residual
    (xb = x + b_out, fp32) is injected into the same psum group by a leading
    identity matmul (2^K*I, fp32r) so the final psum->sbuf pass is a single
    multiply by 2^-K.

All psum->sbuf evacuation passes use [128,1024] (2-bank) tiles to halve the
per-instruction overhead on DVE/ACT, which HW timing showed dominates.
"""
import sys

for _p in (
    "/opt/trn_rl_repo",
    "/root/.axon_site",
    "/root/.axon_site/_ro/trn_rl_repo",
    "/root/.axon_site/_ro/pypackages",
):
    if _p not in sys.path:
        sys.path.append(_p)

import numpy as np
import ml_dtypes

B = 16
C = 512
S = 1024  # H*W
NH = 4
DK = 128
F = NH * DK  # 512
NCORES = 8
BL = B // NCORES  # batches per core
KT = C // 128  # 4 contraction tiles over channels
ST = S // 128  # 8 seq tiles
NT = S // 512  # 2 free-dim chunks of 512
SCALE = float(DK) ** -0.5

M_EXP = 2          # exp computed as exp(t - M_EXP*ln2); 2^M_EXP folded into Z
P_VSC = 7          # v_sc = V * 2^P_VSC / Z
K_RES = 6          # resT scaled 2^(K_RES-P_VSC); residual via 2^K_RES*I matmul
SCH_HEADS = ()     # unused (see SCH_PAIRS)
SCH_PAIRS = ()     # (h, u) jt-pairs whose exp runs on DVE (Schraudolph e5m2)
ACT_CONV_TILES = ()  # qk f-tiles whose psum->bf16 conversion runs on ACT
OUT_CONV_ACT = ()  # out-proj c-tiles whose psum->f32 conversion runs on ACT
RES_CONV_ACT = ()  # heads whose racc->resT conversion runs on ACT
VSC_POOL = True    # v_sc scaling on Pool engine (False: DVE)
USE_FEEDBACK = False  # QK proj xr8 correction: off (PE pacing gates ACT; -9us)
NO_ACCUM_PROBE = False  # TIMING PROBE ONLY: drop accum_out (wrong results)
NO_EXP_PROBE = False    # TIMING PROBE ONLY: skip exp entirely (wrong results)
SCH_A = float(4.0 / np.log(2.0) * SCALE)
SCH_B = float(4.0 * (15 - M_EXP) - 0.45 + 0.5)  # +0.5: DVE int8 convert truncates

_CACHE: dict = {}


def _build(repeat=1):
    """Build the kernel. repeat>1 wraps the per-call workload in an on-device
    For_i loop (timing only; amortizes the ~10ms axon dispatch)."""
    import contextlib

    import concourse.tile as tile
    from concourse import bacc, mybir

    F32 = mybir.dt.float32
    F32R = mybir.dt.float32r
    BF16 = mybir.dt.bfloat16
    E4 = mybir.dt.float8e4
    E5 = mybir.dt.float8e5
    I8 = mybir.dt.int8

    nc = bacc.Bacc("TRN2", debug=False)
    x8_d = nc.dram_tensor("x8", [BL, C, S], E4, kind="ExternalInput").ap()
    xr8_d = nc.dram_tensor("xr8", [BL, C, S], E4, kind="ExternalInput").ap()
    xb_d = nc.dram_tensor("xb", [BL, C, S], F32, kind="ExternalInput").ap()
    xb_r = xb_d.bitcast(F32R)
    wqk_d = nc.dram_tensor("wqk8", [C, 2 * F], E4, kind="ExternalInput").ap()
    wv_d = nc.dram_tensor("wv8", [C, F], E4, kind="ExternalInput").ap()
    wo_d = nc.dram_tensor("wo8", [F, C], E4, kind="ExternalInput").ap()
    NCONST = 2 * NH + 2 * F + 1 + 128
    consts_d = nc.dram_tensor(
        "consts", [128, NCONST], F32, kind="ExternalInput"
    ).ap().bitcast(F32R)
    out_d = nc.dram_tensor("out", [BL, C, S], F32, kind="ExternalOutput").ap()

    wqk_r = wqk_d.rearrange("(k p) m -> p k m", p=128)
    wv_r = wv_d.rearrange("(k p) m -> p k m", p=128)
    wo_r = wo_d.rearrange("(k p) m -> p k m", p=128)

    with tile.TileContext(nc) as tc:
        with (
            tc.tile_pool(name="const", bufs=1) as constp,
            tc.tile_pool(name="xp", bufs=2) as xp,
            tc.tile_pool(name="qkp", bufs=2) as qkp,
            tc.tile_pool(name="vp", bufs=2) as vp,
            tc.tile_pool(name="ep", bufs=6) as ep,
            tc.tile_pool(name="scr", bufs=2) as scrp,
            tc.tile_pool(name="rp", bufs=2) as rp,
            tc.tile_pool(name="op", bufs=2) as op,
            tc.tile_pool(name="small", bufs=8) as smallp,
            tc.tile_pool(name="vs", bufs=8) as vsp,
            # psum: one pool, per-tag rings: "wp" (proj/out-proj) 1x[128,1024],
            # "sacc" 2x[128,1024], "racc" 1x[128,1024] -> 2+4+2 = 8 banks.
            tc.tile_pool(name="pz", bufs=1, space="PSUM") as pz,
        ):
            wqk_sb = constp.tile([128, KT, 2 * F], E4)
            wv_sb = constp.tile([128, KT, F], E4)
            wo_sb = constp.tile([128, KT, C], E4)
            consts_sb = constp.tile([128, NCONST], F32R)
            x8_sbs = [xp.tile([128, KT, S], E4, name=f"x8_{b}", tag="x8") for b in range(BL)]
            xr8_sbs = [xp.tile([128, KT, S], E4, name=f"xr8_{b}", tag="xr8") for b in range(BL)]
            xb_sbs = [xp.tile([128, KT, S], F32R, name=f"xb_{b}", tag="xb") for b in range(BL)]

            # DMA order: first batch's x8 + qk weights first (gate the first
            # matmuls), then the rest. One DMA per (tensor, batch) via 3D APs.
            def xre(ap_d, b):
                return ap_d[b].rearrange("(k p) s -> p k s", p=128)

            nc.sync.dma_start(out=x8_sbs[0], in_=xre(x8_d, 0))
            nc.sync.dma_start(out=wqk_sb, in_=wqk_r)
            nc.sync.dma_start(out=consts_sb, in_=consts_d)
            nc.sync.dma_start(out=xr8_sbs[0], in_=xre(xr8_d, 0))
            nc.sync.dma_start(out=wv_sb, in_=wv_r)
            for b in range(1, BL):
                nc.sync.dma_start(out=x8_sbs[b], in_=xre(x8_d, b))
                nc.sync.dma_start(out=xr8_sbs[b], in_=xre(xr8_d, b))
            nc.sync.dma_start(out=wo_sb, in_=wo_r)
            for b in range(BL):
                nc.sync.dma_start(out=xb_sbs[b], in_=xre(xb_r, b))

            b_qk = consts_sb[:, 0 : 2 * NH].bitcast(F32)
            bv2_sb = consts_sb[:, 2 * NH : 2 * NH + 2 * F].bitcast(F32)
            ebias = consts_sb[:, 2 * NH + 2 * F : 2 * NH + 2 * F + 1].bitcast(F32)
            ident = consts_sb[:, 2 * NH + 2 * F + 1 :]  # [128,128] = 2^K_RES*I (fp32r)

            rep_ctx = tc.For_i(0, repeat, 1) if repeat > 1 else contextlib.nullcontext()
            post_loop = []
            with rep_ctx:
                _batches(
                    nc, tc, x8_sbs, xr8_sbs, xb_sbs, qkp, vp, ep, scrp, rp, op,
                    smallp, vsp, pz, wqk_sb, wv_sb, wo_sb, b_qk, bv2_sb,
                    ebias, ident, out_d, F32, F32R, BF16, E4, E5, I8, mybir,
                    defer_tail=(repeat > 1), post_loop=post_loop,
                )
            for f in post_loop:
                f()

    nc.compile()
    return nc


def _batches(
    nc, tc, x8_sbs, xr8_sbs, xb_sbs, qkp, vp, ep, scrp, rp, op, smallp, vsp,
    pz, wqk_sb, wv_sb, wo_sb, b_qk, bv2_sb, ebias, ident, out_d,
    F32, F32R, BF16, E4, E5, I8, mybir, defer_tail=False, post_loop=None,
):
    """Flat (batch, head) stage pipeline: stage s emits scores+exp for its own
    head and the AV/normalization for stage s-1, so the ACT engine (the
    bottleneck: 64 exp passes) never waits on the softmax->AV->resT chain."""
    DR = mybir.MatmulPerfMode.DoubleRow
    EXP = mybir.ActivationFunctionType.Exp
    IDENT = mybir.ActivationFunctionType.Identity
    MULT = mybir.AluOpType.mult
    ADD = mybir.AluOpType.add

    bctx: dict = {}

    def get_bctx(b):
        if b not in bctx:
            qk_sb = qkp.tile([128, 2 * NH, S], BF16, name=f"qk_{b}", tag="qk")
            v_sb = vp.tile([128, ST // 2, 2 * F], BF16, name=f"v_{b}", tag="v")
            resT_sb = rp.tile([128, NH, S], E4, name=f"resT_{b}", tag="resT")
            bctx[b] = (qk_sb, v_sb, resT_sb)
        return bctx[b]

    def qk_proj(b, t):
        # Q^T/K^T f-tile t: qk[:, t, s] = w_qk[:, t].T @ (x8 + xr8)
        qk_sb = get_bctx(b)[0]
        acc = pz.tile([128, S], F32, name="qka", tag="wp", bufs=1)
        srcs = (x8_sbs[b], xr8_sbs[b]) if USE_FEEDBACK else (x8_sbs[b],)
        for kp in range(KT // 2):
            for si, src in enumerate(srcs):
                for n in range(NT):
                    nc.tensor.matmul(
                        acc[:, bass_ts(n, 512)],
                        wqk_sb[:, 2 * kp : 2 * kp + 2, bass_ts(t, 128)],
                        src[:, 2 * kp : 2 * kp + 2, bass_ts(n, 512)],
                        start=(kp == 0 and si == 0),
                        stop=(kp == KT // 2 - 1 and si == len(srcs) - 1),
                        perf_mode=DR,
                    )
        if t in ACT_CONV_TILES:
            nc.scalar.activation(
                out=qk_sb[:, t, :], in_=acc, func=IDENT, bias=b_qk[:, t : t + 1]
            )
        else:
            nc.vector.tensor_scalar_add(qk_sb[:, t, :], acc, b_qk[:, t : t + 1])

    def v_proj(b, u):
        # V rows for st-pair u: v_sb[:, u, i*F + f] = V[j=128*(2u+i)+p, f]
        v_sb = get_bctx(b)[1]
        acc = pz.tile([128, S], F32, name="vacc", tag="wp", bufs=1)
        for i in range(2):
            st = 2 * u + i
            for kp in range(KT // 2):
                nc.tensor.matmul(
                    acc[:, bass_ts(i, 512)],
                    x8_sbs[b][:, 2 * kp : 2 * kp + 2, bass_ts(st, 128)],
                    wv_sb[:, 2 * kp : 2 * kp + 2, :],
                    start=(kp == 0),
                    stop=(kp == KT // 2 - 1),
                    perf_mode=DR,
                )
        nc.vector.tensor_tensor(v_sb[:, u, :], acc, bv2_sb, op=ADD)

    def out_proj(b, ct):
        # psum = 2^K*xb + wo8.T @ resT; out = psum * 2^-K
        resT_sb = get_bctx(b)[2]
        out_t = op.tile([128, S], F32, name="out_t", tag="out")
        acc = pz.tile([128, S], F32, name="oacc", tag="wp", bufs=1)
        for n in range(NT):
            nc.tensor.matmul(
                acc[:, bass_ts(n, 512)],
                ident,
                xb_sbs[b][:, ct, bass_ts(n, 512)],
                start=True,
                stop=False,
            )
            for kp in range(KT // 2):
                nc.tensor.matmul(
                    acc[:, bass_ts(n, 512)],
                    wo_sb[:, 2 * kp : 2 * kp + 2, bass_ts(ct, 128)],
                    resT_sb[:, 2 * kp : 2 * kp + 2, bass_ts(n, 512)],
                    start=False,
                    stop=(kp == KT // 2 - 1),
                    perf_mode=DR,
                )
        if ct in OUT_CONV_ACT:
            nc.scalar.mul(out_t, acc, float(2.0 ** (-K_RES)))
        else:
            nc.vector.tensor_scalar(out_t, acc, float(2.0 ** (-K_RES)), None, MULT)
        nc.sync.dma_start(
            out=out_d[b, :, :].rearrange("(k p) s -> p k s", p=128)[:, ct, :],
            in_=out_t,
        )

    # ---- static filler schedule: PE-side proj/out work interleaved at the
    # u-granularity of each stage so the wide-psum ring never stalls PE.
    NS = BL * NH
    fillers = {s: [[] for _ in range(4)] for s in range(NS)}
    epilogue = []
    for b in range(BL):
        s0 = b * NH
        if b > 0:
            fillers[s0 - 1][2].append(lambda b=b: qk_proj(b, 0))
            fillers[s0 - 1][3].append(lambda b=b: qk_proj(b, 1))
        for u in range(4):
            fillers[s0][u].append(lambda b=b, u=u: v_proj(b, u))
        fillers[s0][1].append(lambda b=b: qk_proj(b, 2))
        fillers[s0][2].append(lambda b=b: qk_proj(b, 3))
        for h in (1, 2):
            fillers[s0 + h][1].append(lambda b=b, h=h: qk_proj(b, 2 * h + 2))
            fillers[s0 + h][3].append(lambda b=b, h=h: qk_proj(b, 2 * h + 3))
        for ct in range(KT):
            tgt = s0 + NH + 1 + ct // 2
            if tgt < NS:
                fillers[tgt][(ct % 2) * 2].append(lambda b=b, ct=ct: out_proj(b, ct))
            elif defer_tail:
                # emit into the NEXT loop iteration's early stages (reads the
                # previous iteration's resT ring slot); the post-loop drain
                # below produces the final correct output.
                fillers[tgt - NS][(ct % 2) * 2 + 1].append(
                    lambda b=b, ct=ct: out_proj(b, ct)
                )
                post_loop.append(lambda b=b, ct=ct: out_proj(b, ct))
            else:
                epilogue.append(lambda b=b, ct=ct: out_proj(b, ct))

    def av_stage(st, u_pair):
        # normalization + AV for stage st, pairs u_pair (emitted from the
        # following stage so it overlaps that stage's exps)
        b, h, e8s, ssum, racc = st
        v_sb = get_bctx(b)[1]
        for u in u_pair:
            sch = (h, u) in SCH_PAIRS
            v_sc = vsp.tile([128, 2, DK], E4, name="vsc", tag="vsc")
            vsc_eng = nc.gpsimd if VSC_POOL else nc.vector
            for i in range(2):
                jt = 2 * u + i
                vsc_eng.tensor_scalar(
                    v_sc[:, i, :],
                    v_sb[:, u, bass_ts(i, 512)][:, bass_ts(h, DK)],
                    ssum[:, ST + jt : ST + jt + 1], float(2.0 ** P_VSC),
                    MULT, MULT,
                )
            e8 = e8s[u]
            for n in range(NT):
                rhs = e8[:, :, bass_ts(n, 512)]
                nc.tensor.matmul(
                    racc[:, bass_ts(n, 512)],
                    v_sc,
                    rhs.bitcast(E5) if sch else rhs,
                    start=(u == 0),
                    stop=(u == ST // 2 - 1),
                    perf_mode=DR,
                )

    def res_stage(st):
        b, h, e8s, ssum, racc = st
        resT_sb = get_bctx(b)[2]
        if h in RES_CONV_ACT:
            nc.scalar.mul(resT_sb[:, h, :], racc, float(2.0 ** (K_RES - P_VSC)))
        else:
            nc.vector.tensor_scalar(
                resT_sb[:, h, :], racc, float(2.0 ** (K_RES - P_VSC)), None, MULT
            )

    qk_proj(0, 0)
    qk_proj(0, 1)

    prev = None
    for s in range(NS):
        b, h = divmod(s, NH)
        qk_sb = get_bctx(b)[0]
        ssum = smallp.tile([128, 2 * ST], F32, name="ssum", tag="ssum")
        if NO_ACCUM_PROBE or NO_EXP_PROBE:
            nc.vector.memset(ssum[:, 0:ST], 1.0)
        e8s = []
        for u in range(ST // 2):
            sch = (h, u) in SCH_PAIRS
            e8 = ep.tile([128, 2, S], E4, name=f"e8_{u}", tag="e8")
            e8s.append(e8)
            for i in range(2):
                jt = 2 * u + i
                sacc = pz.tile([128, S], F32, name="sacc", tag="sacc", bufs=2)
                for n in range(NT):
                    nc.tensor.matmul(
                        sacc[:, bass_ts(n, 512)],
                        qk_sb[:, 2 * h + 1, bass_ts(jt, 128)],
                        qk_sb[:, 2 * h, bass_ts(n, 512)],
                        start=True,
                        stop=True,
                    )
                if sch:
                    nc.vector.tensor_scalar(
                        e8[:, i, :].bitcast(I8), sacc, SCH_A, SCH_B, MULT, ADD
                    )
                    scr = scrp.tile([128, S], E5, name="scr", tag="scr")
                    nc.vector.tensor_scalar(
                        scr, e8[:, i, :].bitcast(E5), 1.0, None, MULT, ADD,
                        accum_out=ssum[:, jt : jt + 1],
                    )
                elif NO_EXP_PROBE:
                    pass
                elif NO_ACCUM_PROBE:
                    nc.scalar.activation(
                        out=e8[:, i, :], in_=sacc, func=EXP, scale=SCALE,
                        bias=ebias[:, 0:1],
                    )
                else:
                    nc.scalar.activation(
                        out=e8[:, i, :], in_=sacc, func=EXP, scale=SCALE,
                        bias=ebias[:, 0:1], accum_out=ssum[:, jt : jt + 1],
                    )
            for f in fillers[s][u]:
                f()
            if prev is not None:
                if u == 1:
                    av_stage(prev, (0, 1))
                elif u == 3:
                    av_stage(prev, (2, 3))
        if prev is not None:
            res_stage(prev)
        # normalizer for this stage (consumed by av_stage from stage s+1)
        nc.vector.reciprocal(ssum[:, ST : 2 * ST], ssum[:, 0:ST])
        racc = pz.tile([128, S], F32, name="racc", tag="racc", bufs=1)
        prev = (b, h, e8s, ssum, racc)

    av_stage(prev, (0, 1))
    av_stage(prev, (2, 3))
    res_stage(prev)
    for f in epilogue:
        f()


def bass_ts(i, size):
    import concourse.bass as bass

    return bass.ts(i, size)


def _prep_inputs(x, w_proj, b_proj, w_out, b_out):
    """Host-side quantization + reshaping into the layouts the kernel expects."""
    E4 = ml_dtypes.float8_e4m3
    x_f = np.ascontiguousarray(np.asarray(x, np.float32).reshape(B, C, S))
    b_out = np.asarray(b_out, np.float32)
    xb = np.ascontiguousarray(x_f + b_out[None, :, None])
    x8 = x_f.astype(E4)
    xr8 = np.ascontiguousarray(x_f - x8.astype(np.float32)).astype(E4)
    x8 = np.ascontiguousarray(x8)

    wT = np.asarray(w_proj, np.float32).T  # [C, 3F]
    w_qkT = np.concatenate(
        [wT[:, h * 384 : h * 384 + 256] for h in range(NH)], axis=1
    )  # [C, 2F]; col tile t=2h -> q_h, t=2h+1 -> k_h
    w_vT = np.concatenate(
        [wT[:, h * 384 + 256 : h * 384 + 384] for h in range(NH)], axis=1
    )  # [C, F]
    w_outT = np.asarray(w_out, np.float32).T  # [F, C]
    wqk8 = np.ascontiguousarray(w_qkT).astype(E4)
    wv8 = np.ascontiguousarray(w_vT).astype(E4)
    wo8 = np.ascontiguousarray(w_outT).astype(E4)

    b_proj = np.asarray(b_proj, np.float32)
    b_qk = np.stack(
        [
            b_proj[h * 384 + half * 128 : h * 384 + half * 128 + 128]
            for h in range(NH)
            for half in range(2)
        ],
        axis=1,
    )  # [128, 2*NH]
    b_v = np.concatenate([b_proj[h * 384 + 256 : h * 384 + 384] for h in range(NH)])
    bv2 = np.broadcast_to(np.concatenate([b_v, b_v]), (128, 2 * F))
    eb = np.full((128, 1), -M_EXP * np.log(2.0), np.float32)
    ident = np.eye(128, dtype=np.float32) * float(2.0 ** K_RES)
    consts = np.ascontiguousarray(
        np.concatenate([b_qk, bv2, eb, ident], axis=1), dtype=np.float32
    )
    return x8, xr8, xb, wqk8, wv8, wo8, consts


def kernel(x, w_proj, b_proj, w_out, b_out, n_heads):
    from concourse.bass_utils import run_bass_kernel_spmd

    assert int(n_heads) == NH
    x8, xr8, xb, wqk8, wv8, wo8, consts = _prep_inputs(x, w_proj, b_proj, w_out, b_out)

    if "nc" not in _CACHE:
        _CACHE["nc"] = _build()
    nc = _CACHE["nc"]

    in_maps = [
        {
            "x8": np.ascontiguousarray(x8[c * BL : (c + 1) * BL]),
            "xr8": np.ascontiguousarray(xr8[c * BL : (c + 1) * BL]),
            "xb": np.ascontiguousarray(xb[c * BL : (c + 1) * BL]),
            "wqk8": wqk8,
            "wv8": wv8,
            "wo8": wo8,
            "consts": consts,
        }
        for c in range(NCORES)
    ]
    res = run_bass_kernel_spmd(nc, in_maps, list(range(NCORES)))
    out = np.concatenate([res.results[c]["out"] for c in range(NCORES)], axis=0)
    return out.reshape(B, C, 32, 32)


# revision 17
# speedup vs baseline: 1.4260x; 1.0894x over previous
"""AttentionBlock Trainium2 kernel (fp8 DoubleRow + multi-engine softmax).

Reference computation (B=16, C=512, H=W=32, n_heads=4, d_k=128):
    xs   = x.reshape(B,C,S).T            # [B, S, C],  S = 1024
    qkv  = xs @ w_proj.T + b_proj        # [B, S, 1536]
    S_   = einsum('bihd,bjhd->bijh', q, k) * d_k**-0.5
    attn = softmax(S_, axis=1)           # over the QUERY axis i (source quirk)
    res  = einsum('bijh,bjhd->bihd', attn, v)
    out  = res @ w_out.T + b_out + xs    # residual
    return out.T.reshape(B, C, H, W)

Strategy: data-parallel over batch, 2 batches per core on 8 cores.

Dtype plan (validated vs the reference in numpy, rel err ~9.4e-3 < 2e-2):
  - QKV projections: e4m3 x & weights, DoubleRow fp8 matmuls (2 k-tiles of 128
    contracted per MM, 2 output cols/cycle). QK additionally accumulates an
    x-residual correction term (xr8 = e4m3(x - e4m3(x))) to halve proj noise.
  - Q,K stored bf16; scores matmul bf16 (contraction d_k=128, no DR possible).
  - exp on ACT: scale=dk**-0.5, bias=-2*ln2 keeps e4m3 in range; accum_out
    produces the softmax-axis sums. (Optionally SCH_HEADS run exp on DVE via a
    Schraudolph bit-trick in e5m2; measured slower on HW - off by default.)
  - AV: DoubleRow fp8, stationary v_sc = e4m3(V * 2^P / Z_j) (Pool engine),
    moving e8 tiles.
  - out-proj: DoubleRow e4m3 (resT stored e4m3 scaled 2^(K-P)); the residual
    (xb = x + b_out, fp32) is injected into the same psum group by a leading
    identity matmul (2^K*I, fp32r) so the final psum->sbuf pass is a single
    multiply by 2^-K.

All psum->sbuf evacuation passes use [128,1024] (2-bank) tiles to halve the
per-instruction overhead on DVE/ACT, which HW timing showed dominates.
"""
import sys

for _p in (
    "/opt/trn_rl_repo",
    "/root/.axon_site",
    "/root/.axon_site/_ro/trn_rl_repo",
    "/root/.axon_site/_ro/pypackages",
):
    if _p not in sys.path:
        sys.path.append(_p)

import numpy as np
import ml_dtypes

B = 16
C = 512
S = 1024  # H*W
NH = 4
DK = 128
F = NH * DK  # 512
NCORES = 8
BL = B // NCORES  # batches per core
KT = C // 128  # 4 contraction tiles over channels
ST = S // 128  # 8 seq tiles
NT = S // 512  # 2 free-dim chunks of 512
SCALE = float(DK) ** -0.5

M_EXP = 2          # exp computed as exp(t - M_EXP*ln2); 2^M_EXP folded into Z
P_VSC = 7          # v_sc = V * 2^P_VSC / Z
K_RES = 6          # resT scaled 2^(K_RES-P_VSC); residual via 2^K_RES*I matmul
SCH_HEADS = ()     # unused (see SCH_PAIRS)
SCH_PAIRS = ()     # (h, u) jt-pairs whose exp runs on DVE (Schraudolph e5m2)
ACT_CONV_TILES = ()  # qk f-tiles whose psum->bf16 conversion runs on ACT
OUT_CONV_ACT = ()  # out-proj c-tiles whose psum->f32 conversion runs on ACT
RES_CONV_ACT = ()  # heads whose racc->resT conversion runs on ACT
VSC_POOL = True    # v_sc scaling on Pool engine (False: DVE)
USE_FEEDBACK = False  # QK proj xr8 correction: off (PE pacing gates ACT; -9us)
NO_ACCUM_PROBE = False  # TIMING PROBE ONLY: drop accum_out (wrong results)
NO_EXP_PROBE = False    # TIMING PROBE ONLY: skip exp entirely (wrong results)
SCH_A = float(4.0 / np.log(2.0) * SCALE)
SCH_B = float(4.0 * (15 - M_EXP) - 0.45 + 0.5)  # +0.5: DVE int8 convert truncates

_CACHE: dict = {}


def _build(repeat=1):
    """Build the kernel. repeat>1 wraps the per-call workload in an on-device
    For_i loop (timing only; amortizes the ~10ms axon dispatch)."""
    import contextlib

    import concourse.tile as tile
    from concourse import bacc, mybir

    F32 = mybir.dt.float32
    F32R = mybir.dt.float32r
    BF16 = mybir.dt.bfloat16
    E4 = mybir.dt.float8e4
    E5 = mybir.dt.float8e5
    I8 = mybir.dt.int8

    nc = bacc.Bacc("TRN2", debug=False)
    x8_d = nc.dram_tensor("x8", [BL, C, S], E4, kind="ExternalInput").ap()
    xr8_d = nc.dram_tensor("xr8", [BL, C, S], E4, kind="ExternalInput").ap()
    xb_d = nc.dram_tensor("xb", [BL, C, S], F32, kind="ExternalInput").ap()
    xb_r = xb_d.bitcast(F32R)
    wqk_d = nc.dram_tensor("wqk8", [C, 2 * F], E4, kind="ExternalInput").ap()
    wv_d = nc.dram_tensor("wv8", [C, F], E4, kind="ExternalInput").ap()
    wo_d = nc.dram_tensor("wo8", [F, C], E4, kind="ExternalInput").ap()
    NCONST = 2 * NH + 2 * F + 1 + 128
    consts_d = nc.dram_tensor(
        "consts", [128, NCONST], F32, kind="ExternalInput"
    ).ap().bitcast(F32R)
    out_d = nc.dram_tensor("out", [BL, C, S], F32, kind="ExternalOutput").ap()

    wqk_r = wqk_d.rearrange("(k p) m -> p k m", p=128)
    wv_r = wv_d.rearrange("(k p) m -> p k m", p=128)
    wo_r = wo_d.rearrange("(k p) m -> p k m", p=128)

    with tile.TileContext(nc) as tc:
        with (
            tc.tile_pool(name="const", bufs=1) as constp,
            tc.tile_pool(name="xp", bufs=2) as xp,
            tc.tile_pool(name="qkp", bufs=2) as qkp,
            tc.tile_pool(name="vp", bufs=2) as vp,
            tc.tile_pool(name="ep", bufs=10) as ep,
            tc.tile_pool(name="scr", bufs=2) as scrp,
            tc.tile_pool(name="rp", bufs=2) as rp,
            tc.tile_pool(name="op", bufs=2) as op,
            tc.tile_pool(name="small", bufs=8) as smallp,
            tc.tile_pool(name="vs", bufs=8) as vsp,
            # psum: one pool, per-tag rings: "wp" (proj/out-proj) 1x[128,1024],
            # "sacc" 2x[128,1024], "racc" 1x[128,1024] -> 2+4+2 = 8 banks.
            tc.tile_pool(name="pz", bufs=1, space="PSUM") as pz,
        ):
            wqk_sb = constp.tile([128, KT, 2 * F], E4)
            wv_sb = constp.tile([128, KT, F], E4)
            wo_sb = constp.tile([128, KT, C], E4)
            consts_sb = constp.tile([128, NCONST], F32R)
            x8_sbs = [xp.tile([128, KT, S], E4, name=f"x8_{b}", tag="x8") for b in range(BL)]
            xr8_sbs = [xp.tile([128, KT, S], E4, name=f"xr8_{b}", tag="xr8") for b in range(BL)]
            xb_sbs = [xp.tile([128, KT, S], F32R, name=f"xb_{b}", tag="xb") for b in range(BL)]

            # DMA order: first batch's x8 + qk weights first (gate the first
            # matmuls), then the rest. One DMA per (tensor, batch) via 3D APs.
            def xre(ap_d, b):
                return ap_d[b].rearrange("(k p) s -> p k s", p=128)

            nc.sync.dma_start(out=x8_sbs[0], in_=xre(x8_d, 0))
            nc.sync.dma_start(out=wqk_sb, in_=wqk_r)
            nc.sync.dma_start(out=consts_sb, in_=consts_d)
            if USE_FEEDBACK:
                nc.sync.dma_start(out=xr8_sbs[0], in_=xre(xr8_d, 0))
            nc.sync.dma_start(out=wv_sb, in_=wv_r)
            for b in range(1, BL):
                nc.sync.dma_start(out=x8_sbs[b], in_=xre(x8_d, b))
                if USE_FEEDBACK:
                    nc.sync.dma_start(out=xr8_sbs[b], in_=xre(xr8_d, b))
            nc.sync.dma_start(out=wo_sb, in_=wo_r)
            for b in range(BL):
                nc.sync.dma_start(out=xb_sbs[b], in_=xre(xb_r, b))

            b_qk = consts_sb[:, 0 : 2 * NH].bitcast(F32)
            bv2_sb = consts_sb[:, 2 * NH : 2 * NH + 2 * F].bitcast(F32)
            ebias = consts_sb[:, 2 * NH + 2 * F : 2 * NH + 2 * F + 1].bitcast(F32)
            ident = consts_sb[:, 2 * NH + 2 * F + 1 :]  # [128,128] = 2^K_RES*I (fp32r)

            rep_ctx = tc.For_i(0, repeat, 1) if repeat > 1 else contextlib.nullcontext()
            post_loop = []
            with rep_ctx:
                _batches(
                    nc, tc, x8_sbs, xr8_sbs, xb_sbs, qkp, vp, ep, scrp, rp, op,
                    smallp, vsp, pz, wqk_sb, wv_sb, wo_sb, b_qk, bv2_sb,
                    ebias, ident, out_d, F32, F32R, BF16, E4, E5, I8, mybir,
                    defer_tail=(repeat > 1), post_loop=post_loop,
                )
            for f in post_loop:
                f()

    nc.compile()
    return nc


def _batches(
    nc, tc, x8_sbs, xr8_sbs, xb_sbs, qkp, vp, ep, scrp, rp, op, smallp, vsp,
    pz, wqk_sb, wv_sb, wo_sb, b_qk, bv2_sb, ebias, ident, out_d,
    F32, F32R, BF16, E4, E5, I8, mybir, defer_tail=False, post_loop=None,
):
    """Flat (batch, head) stage pipeline: stage s emits scores+exp for its own
    head and the AV/normalization for stage s-1, so the ACT engine (the
    bottleneck: 64 exp passes) never waits on the softmax->AV->resT chain."""
    DR = mybir.MatmulPerfMode.DoubleRow
    EXP = mybir.ActivationFunctionType.Exp
    IDENT = mybir.ActivationFunctionType.Identity
    MULT = mybir.AluOpType.mult
    ADD = mybir.AluOpType.add

    bctx: dict = {}

    def get_bctx(b):
        if b not in bctx:
            qk_sb = qkp.tile([128, 2 * NH, S], BF16, name=f"qk_{b}", tag="qk")
            v_sb = vp.tile([128, ST // 2, 2 * F], BF16, name=f"v_{b}", tag="v")
            resT_sb = rp.tile([128, NH, S], E4, name=f"resT_{b}", tag="resT")
            bctx[b] = (qk_sb, v_sb, resT_sb)
        return bctx[b]

    def qk_proj(b, t):
        # Q^T/K^T f-tile t: qk[:, t, s] = w_qk[:, t].T @ (x8 + xr8)
        qk_sb = get_bctx(b)[0]
        acc = pz.tile([128, S], F32, name="qka", tag="wp", bufs=1)
        srcs = (x8_sbs[b], xr8_sbs[b]) if USE_FEEDBACK else (x8_sbs[b],)
        for kp in range(KT // 2):
            for si, src in enumerate(srcs):
                for n in range(NT):
                    nc.tensor.matmul(
                        acc[:, bass_ts(n, 512)],
                        wqk_sb[:, 2 * kp : 2 * kp + 2, bass_ts(t, 128)],
                        src[:, 2 * kp : 2 * kp + 2, bass_ts(n, 512)],
                        start=(kp == 0 and si == 0),
                        stop=(kp == KT // 2 - 1 and si == len(srcs) - 1),
                        perf_mode=DR,
                    )
        if t in ACT_CONV_TILES:
            nc.scalar.activation(
                out=qk_sb[:, t, :], in_=acc, func=IDENT, bias=b_qk[:, t : t + 1]
            )
        else:
            nc.vector.tensor_scalar_add(qk_sb[:, t, :], acc, b_qk[:, t : t + 1])

    def v_proj(b, u):
        # V rows for st-pair u: v_sb[:, u, i*F + f] = V[j=128*(2u+i)+p, f]
        v_sb = get_bctx(b)[1]
        acc = pz.tile([128, S], F32, name="vacc", tag="wp", bufs=1)
        for i in range(2):
            st = 2 * u + i
            for kp in range(KT // 2):
                nc.tensor.matmul(
                    acc[:, bass_ts(i, 512)],
                    x8_sbs[b][:, 2 * kp : 2 * kp + 2, bass_ts(st, 128)],
                    wv_sb[:, 2 * kp : 2 * kp + 2, :],
                    start=(kp == 0),
                    stop=(kp == KT // 2 - 1),
                    perf_mode=DR,
                )
        nc.vector.tensor_tensor(v_sb[:, u, :], acc, bv2_sb, op=ADD)

    def out_proj(b, ct):
        # psum = 2^K*xb + wo8.T @ resT; out = psum * 2^-K
        resT_sb = get_bctx(b)[2]
        out_t = op.tile([128, S], F32, name="out_t", tag="out")
        acc = pz.tile([128, S], F32, name="oacc", tag="wp", bufs=1)
        for n in range(NT):
            nc.tensor.matmul(
                acc[:, bass_ts(n, 512)],
                ident,
                xb_sbs[b][:, ct, bass_ts(n, 512)],
                start=True,
                stop=False,
            )
            for kp in range(KT // 2):
                nc.tensor.matmul(
                    acc[:, bass_ts(n, 512)],
                    wo_sb[:, 2 * kp : 2 * kp + 2, bass_ts(ct, 128)],
                    resT_sb[:, 2 * kp : 2 * kp + 2, bass_ts(n, 512)],
                    start=False,
                    stop=(kp == KT // 2 - 1),
                    perf_mode=DR,
                )
        if ct in OUT_CONV_ACT:
            nc.scalar.mul(out_t, acc, float(2.0 ** (-K_RES)))
        else:
            nc.vector.tensor_scalar(out_t, acc, float(2.0 ** (-K_RES)), None, MULT)
        nc.sync.dma_start(
            out=out_d[b, :, :].rearrange("(k p) s -> p k s", p=128)[:, ct, :],
            in_=out_t,
        )

    # ---- static filler schedule: PE-side proj/out work interleaved at the
    # u-granularity of each stage so the wide-psum ring never stalls PE.
    NS = BL * NH
    fillers = {s: [[] for _ in range(4)] for s in range(NS)}
    epilogue = []
    for b in range(BL):
        s0 = b * NH
        if b > 0:
            fillers[s0 - 1][2].append(lambda b=b: qk_proj(b, 0))
            fillers[s0 - 1][3].append(lambda b=b: qk_proj(b, 1))
        for u in range(4):
            fillers[s0][u].append(lambda b=b, u=u: v_proj(b, u))
        fillers[s0][1].append(lambda b=b: qk_proj(b, 2))
        fillers[s0][2].append(lambda b=b: qk_proj(b, 3))
        for h in (1, 2):
            fillers[s0 + h][1].append(lambda b=b, h=h: qk_proj(b, 2 * h + 2))
            fillers[s0 + h][3].append(lambda b=b, h=h: qk_proj(b, 2 * h + 3))
        for ct in range(KT):
            tgt = s0 + NH + 1 + ct // 2
            if tgt < NS:
                fillers[tgt][(ct % 2) * 2].append(lambda b=b, ct=ct: out_proj(b, ct))
            elif defer_tail:
                # emit into the NEXT loop iteration's early stages (reads the
                # previous iteration's resT ring slot); the post-loop drain
                # below produces the final correct output.
                fillers[tgt - NS][(ct % 2) * 2 + 1].append(
                    lambda b=b, ct=ct: out_proj(b, ct)
                )
                post_loop.append(lambda b=b, ct=ct: out_proj(b, ct))
            else:
                epilogue.append(lambda b=b, ct=ct: out_proj(b, ct))

    def av_stage(st, u_pair):
        # normalization + AV for stage st, pairs u_pair (emitted from the
        # following stage so it overlaps that stage's exps)
        b, h, e8s, ssum, racc = st
        v_sb = get_bctx(b)[1]
        for u in u_pair:
            sch = (h, u) in SCH_PAIRS
            v_sc = vsp.tile([128, 2, DK], E4, name="vsc", tag="vsc")
            vsc_eng = nc.gpsimd if VSC_POOL else nc.vector
            for i in range(2):
                jt = 2 * u + i
                vsc_eng.tensor_scalar(
                    v_sc[:, i, :],
                    v_sb[:, u, bass_ts(i, 512)][:, bass_ts(h, DK)],
                    ssum[:, ST + jt : ST + jt + 1], float(2.0 ** P_VSC),
                    MULT, MULT,
                )
            e8 = e8s[u]
            for n in range(NT):
                rhs = e8[:, :, bass_ts(n, 512)]
                nc.tensor.matmul(
                    racc[:, bass_ts(n, 512)],
                    v_sc,
                    rhs.bitcast(E5) if sch else rhs,
                    start=(u == 0),
                    stop=(u == ST // 2 - 1),
                    perf_mode=DR,
                )

    def res_stage(st):
        b, h, e8s, ssum, racc = st
        resT_sb = get_bctx(b)[2]
        if h in RES_CONV_ACT:
            nc.scalar.mul(resT_sb[:, h, :], racc, float(2.0 ** (K_RES - P_VSC)))
        else:
            nc.vector.tensor_scalar(
                resT_sb[:, h, :], racc, float(2.0 ** (K_RES - P_VSC)), None, MULT
            )

    qk_proj(0, 0)
    qk_proj(0, 1)

    prev = None
    for s in range(NS):
        b, h = divmod(s, NH)
        qk_sb = get_bctx(b)[0]
        ssum = smallp.tile([128, 2 * ST], F32, name="ssum", tag="ssum")
        if NO_ACCUM_PROBE or NO_EXP_PROBE:
            nc.vector.memset(ssum[:, 0:ST], 1.0)
        e8s = []
        for u in range(ST // 2):
            sch = (h, u) in SCH_PAIRS
            e8 = ep.tile([128, 2, S], E4, name=f"e8_{u}", tag="e8")
            e8s.append(e8)
            for i in range(2):
                jt = 2 * u + i
                sacc = pz.tile([128, S], F32, name="sacc", tag="sacc", bufs=2)
                for n in range(NT):
                    nc.tensor.matmul(
                        sacc[:, bass_ts(n, 512)],
                        qk_sb[:, 2 * h + 1, bass_ts(jt, 128)],
                        qk_sb[:, 2 * h, bass_ts(n, 512)],
                        start=True,
                        stop=True,
                    )
                if sch:
                    nc.vector.tensor_scalar(
                        e8[:, i, :].bitcast(I8), sacc, SCH_A, SCH_B, MULT, ADD
                    )
                    scr = scrp.tile([128, S], E5, name="scr", tag="scr")
                    nc.vector.tensor_scalar(
                        scr, e8[:, i, :].bitcast(E5), 1.0, None, MULT, ADD,
                        accum_out=ssum[:, jt : jt + 1],
                    )
                elif NO_EXP_PROBE:
                    pass
                elif NO_ACCUM_PROBE:
                    nc.scalar.activation(
                        out=e8[:, i, :], in_=sacc, func=EXP, scale=SCALE,
                        bias=ebias[:, 0:1],
                    )
                else:
                    nc.scalar.activation(
                        out=e8[:, i, :], in_=sacc, func=EXP, scale=SCALE,
                        bias=ebias[:, 0:1], accum_out=ssum[:, jt : jt + 1],
                    )
            for f in fillers[s][u]:
                f()
            if prev is not None:
                if u == 1:
                    av_stage(prev, (0, 1))
                elif u == 3:
                    av_stage(prev, (2, 3))
        if prev is not None:
            res_stage(prev)
        # normalizer for this stage (consumed by av_stage from stage s+1)
        nc.vector.reciprocal(ssum[:, ST : 2 * ST], ssum[:, 0:ST])
        racc = pz.tile([128, S], F32, name="racc", tag="racc", bufs=1)
        prev = (b, h, e8s, ssum, racc)

    av_stage(prev, (0, 1))
    av_stage(prev, (2, 3))
    res_stage(prev)
    for f in epilogue:
        f()


def bass_ts(i, size):
    import concourse.bass as bass

    return bass.ts(i, size)


def _prep_inputs(x, w_proj, b_proj, w_out, b_out):
    """Host-side quantization + reshaping into the layouts the kernel expects."""
    E4 = ml_dtypes.float8_e4m3
    x_f = np.ascontiguousarray(np.asarray(x, np.float32).reshape(B, C, S))
    b_out = np.asarray(b_out, np.float32)
    xb = np.ascontiguousarray(x_f + b_out[None, :, None])
    x8 = x_f.astype(E4)
    xr8 = np.ascontiguousarray(x_f - x8.astype(np.float32)).astype(E4)
    x8 = np.ascontiguousarray(x8)

    wT = np.asarray(w_proj, np.float32).T  # [C, 3F]
    w_qkT = np.concatenate(
        [wT[:, h * 384 : h * 384 + 256] for h in range(NH)], axis=1
    )  # [C, 2F]; col tile t=2h -> q_h, t=2h+1 -> k_h
    w_vT = np.concatenate(
        [wT[:, h * 384 + 256 : h * 384 + 384] for h in range(NH)], axis=1
    )  # [C, F]
    w_outT = np.asarray(w_out, np.float32).T  # [F, C]
    wqk8 = np.ascontiguousarray(w_qkT).astype(E4)
    wv8 = np.ascontiguousarray(w_vT).astype(E4)
    wo8 = np.ascontiguousarray(w_outT).astype(E4)

    b_proj = np.asarray(b_proj, np.float32)
    b_qk = np.stack(
        [
            b_proj[h * 384 + half * 128 : h * 384 + half * 128 + 128]
            for h in range(NH)
            for half in range(2)
        ],
        axis=1,
    )  # [128, 2*NH]
    b_v = np.concatenate([b_proj[h * 384 + 256 : h * 384 + 384] for h in range(NH)])
    bv2 = np.broadcast_to(np.concatenate([b_v, b_v]), (128, 2 * F))
    eb = np.full((128, 1), -M_EXP * np.log(2.0), np.float32)
    ident = np.eye(128, dtype=np.float32) * float(2.0 ** K_RES)
    consts = np.ascontiguousarray(
        np.concatenate([b_qk, bv2, eb, ident], axis=1), dtype=np.float32
    )
    return x8, xr8, xb, wqk8, wv8, wo8, consts


def kernel(x, w_proj, b_proj, w_out, b_out, n_heads):
    from concourse.bass_utils import run_bass_kernel_spmd

    assert int(n_heads) == NH
    x8, xr8, xb, wqk8, wv8, wo8, consts = _prep_inputs(x, w_proj, b_proj, w_out, b_out)

    if "nc" not in _CACHE:
        _CACHE["nc"] = _build()
    nc = _CACHE["nc"]

    in_maps = [
        {
            "x8": np.ascontiguousarray(x8[c * BL : (c + 1) * BL]),
            "xr8": np.ascontiguousarray(xr8[c * BL : (c + 1) * BL]),
            "xb": np.ascontiguousarray(xb[c * BL : (c + 1) * BL]),
            "wqk8": wqk8,
            "wv8": wv8,
            "wo8": wo8,
            "consts": consts,
        }
        for c in range(NCORES)
    ]
    res = run_bass_kernel_spmd(nc, in_maps, list(range(NCORES)))
    out = np.concatenate([res.results[c]["out"] for c in range(NCORES)], axis=0)
    return out.reshape(B, C, 32, 32)
